# revision 1
# baseline (speedup 1.0000x reference)
"""Trainium2 Bass kernel for a dense transformer block (self-attn + cross-attn + MLP).

Sharding: data-parallel over batch, one batch element per NeuronCore (B=8, 8 cores),
no collectives. All activations are kept feature-major ([C, T]) on chip so every
projection matmul consumes weight tiles directly from DRAM (lhsT = W[k,m] slabs) and
activations as the moving operand; matmuls run in float32r (full PE rate at moving
dim >= 256, ~1e-4 relative rounding).

Self/cross attention uses the S^T ([keys, queries]) layout: softmax reduction over
keys is the PSUM accumulation direction; the denominator comes free from a ones
column appended to V (row 64 of the AV matmul output); 1/den is broadcast across
partitions with a K=1 ones-matmul on the PE. Causal masking multiplies exp(S^T)
diagonal tiles by slices of one precomputed [128, 896] master mask.

SBUF pools are stack-allocated per side; long-lived tensors (xT, v_aug, q/k, the
residual, u) live on the right-side stack, phase transients on the left.
"""

import sys
import numpy as np

sys.path.insert(0, "/opt/trn_rl_repo")

B, T, C = 8, 1024, 1024
H = 16
D = C // H          # 64
TI = 256
FF = 4 * C          # 4096
EPS = 1e-5
NCT = C // 128      # 8 c tiles
NTT = T // 128      # 8 t tiles
NFT = FF // 128     # 32 ff tiles
P = 128

_CACHED = {}


def _build():
    import concourse.tile as tile
    from concourse import bacc, mybir
    from concourse.masks import make_identity

    F32, F32R = mybir.dt.float32, mybir.dt.float32r
    AF = mybir.ActivationFunctionType
    OP = mybir.AluOpType

    nc = bacc.Bacc("TRN2", target_bir_lowering=False, debug=False, num_devices=8)

    dr = {}
    dr["x"] = nc.dram_tensor("x", [T, C], F32, kind="ExternalInput")
    dr["x_img_feats"] = nc.dram_tensor("x_img_feats", [TI, C], F32, kind="ExternalInput")
    for nm, shp in [
        ("ln1_g", [C]), ("ln1_b", [C]), ("ln2_g", [C]), ("ln2_b", [C]),
        ("W_attn", [C, 3 * C]), ("b_attn", [3 * C]),
        ("W_aproj", [C, C]), ("b_aproj", [C]),
        ("Wq", [C, C]), ("bq", [C]), ("Wk", [C, C]), ("bk", [C]),
        ("Wv", [C, C]), ("bv", [C]), ("Wcproj", [C, C]), ("bcproj", [C]),
        ("W_fc", [C, FF]), ("b_fc", [FF]), ("W_mproj", [FF, C]), ("b_mproj", [C]),
    ]:
        dr[nm] = nc.dram_tensor(nm, shp, F32, kind="ExternalInput")
    out_d = nc.dram_tensor("out", [T, C], F32, kind="ExternalOutput")

    def W2d(name):
        return dr[name].ap()

    with tile.TileContext(nc) as tc, nc.allow_low_precision(
        reason="float32r rounding of matmul operands is intentional"
    ):
        kw_cms = []

        def openp(**kw):
            cm = tc.tile_pool(**kw)
            return cm, cm.__enter__()

        def openkw(**kw):
            cm, p = openp(**kw)
            kw_cms.append(cm)
            return p

        # ---------------- kernel-wide pools (left-stack base) ----------------
        constp = openkw(name="const", bufs=1)
        scrp = openkw(name="scr", bufs=2)       # f32 [128,1024] ln scratch
        fsrp = openkw(name="fsr", bufs=2)       # f32r [128,512] squares
        abp = openkw(name="ab", bufs=1)         # A_b/B_b [128,1024]
        rowp = openkw(name="rows", bufs=5)      # one [1,1024] "row" tag
        rbp = openkw(name="rb", bufs=3)         # [64,512] + [1,512] rden
        dramp = openkw(name="dram", bufs=1, space="DRAM")

        # ---------------- constants ----------------
        ident = constp.tile([P, P], F32)
        make_identity(nc, ident)
        identR = constp.tile([P, P], F32R)
        nc.vector.tensor_copy(out=identR, in_=ident)

        ones_col = constp.tile([P, 16], F32)
        nc.vector.memset(ones_col, 1.0)
        ones128R = constp.tile([P, 1], F32R)
        nc.vector.tensor_copy(out=ones128R, in_=ones_col[:, 0:1])
        o1x = constp.tile([1, P], F32)
        nc.vector.memset(o1x, 1.0)
        ones_1x128 = constp.tile([1, P], F32R)
        nc.vector.tensor_copy(out=ones_1x128, in_=o1x)
        eps_t = constp.tile([1, 1], F32)
        nc.vector.memset(eps_t, EPS)
        zeros384 = constp.tile([P, 384], F32)
        nc.vector.memset(zeros384, 0.0)

        # master causal mask [128, 896]: keep (1.0) iff (col - row - 384) >= 0.
        # slice [:, 384-128j : 896-128j] == keep iff t_local >= s_local + 128*j
        master = constp.tile([P, 896], F32)
        nc.gpsimd.memset(master, 1.0)
        nc.gpsimd.affine_select(
            out=master, in_=master, compare_op=OP.is_ge, fill=0.0,
            base=-384, pattern=[[1, 896]], channel_multiplier=-1)

        def load_cols(name, nf):
            t = constp.tile([P, nf], F32, name=name + "_c")
            nc.sync.dma_start(out=t, in_=dr[name].ap().rearrange("(f p) -> p f", p=P))
            return t

        g1, b1 = load_cols("ln1_g", NCT), load_cols("ln1_b", NCT)
        g2, b2 = load_cols("ln2_g", NCT), load_cols("ln2_b", NCT)
        bqk = constp.tile([P, 16], F32)
        nc.sync.dma_start(out=bqk, in_=dr["b_attn"].ap()[0:2 * C].rearrange("(f p) -> p f", p=P))
        bq_c = load_cols("bq", NCT)
        bk_c = load_cols("bk", NCT)
        bap_c = load_cols("b_aproj", NCT)
        bcp_c = load_cols("bcproj", NCT)
        bmp_c = load_cols("b_mproj", NCT)
        bfc_c = load_cols("b_fc", NFT)

        xT_d = dramp.tile([NCT, P, T], F32R)    # residual spill

        # ---------------- helpers ----------------
        def bcast_row(row_f32, dest_pool, psp, tag):
            """[1, C] f32 row -> [128, C] f32 partition-broadcast tile."""
            rowr = rowp.tile([1, C], F32R, tag="row", name="rowr")
            nc.vector.tensor_copy(out=rowr, in_=row_f32)
            dest = dest_pool.tile([P, C], F32, tag=tag, name=tag)
            for cc in range(2):
                bps = psp.tile([P, 512], F32, tag="bc", name="bc")
                nc.tensor.matmul(bps, ones_1x128, rowr[:, 512 * cc:512 * (cc + 1)],
                                 start=True, stop=True)
                nc.scalar.copy(out=dest[:, 512 * cc:512 * (cc + 1)], in_=bps)
            return dest

        def load_wslab(wap, co, wpool, eng=None):
            """W[:, co*128:(co+1)*128] ([K, 128]) -> f32r [128, K/128, 128]."""
            nk = wap.shape[0] // P
            stage = wpool.tile([P, nk, P], F32, tag="ws", name="ws")
            nc.sync.dma_start(
                out=stage,
                in_=wap[:, co * P:(co + 1) * P].rearrange("(c p) f -> p c f", p=P))
            wr = wpool.tile([P, nk, P], F32R, tag="wr", name="wr")
            (eng or nc.gpsimd).tensor_copy(out=wr, in_=stage)
            return wr

        def ln_stats(xtiles, psp):
            """Feature-dim LN stats for feature-major tiles -> (A_b, B_b) [128,T]
            f32 broadcast tiles with xhat = x*A_b + B_b (A=rstd, B=-mu*rstd)."""
            sum_ps, sq_ps = [], []
            for tch in range(2):
                sp = psp.tile([1, 512], F32, tag="lnsum", name="lnsum")
                qp = psp.tile([1, 512], F32, tag="lnsq", name="lnsq")
                for c in range(NCT):
                    xs = xtiles[c][:, 512 * tch:512 * (tch + 1)]
                    nc.tensor.matmul(sp, ones128R, xs, start=(c == 0), stop=(c == NCT - 1))
                    sq = fsrp.tile([P, 512], F32R, tag="sq", name="sq")
                    nc.vector.tensor_tensor(out=sq, in0=xs, in1=xs, op=OP.mult)
                    nc.tensor.matmul(qp, ones128R, sq, start=(c == 0), stop=(c == NCT - 1))
                sum_ps.append(sp)
                sq_ps.append(qp)
            mu = rowp.tile([1, T], F32, tag="row", name="mu")
            msq = rowp.tile([1, T], F32, tag="row", name="msq")
            for tch in range(2):
                sl = slice(512 * tch, 512 * (tch + 1))
                nc.vector.tensor_scalar_mul(out=mu[:, sl], in0=sum_ps[tch], scalar1=1.0 / C)
                nc.vector.tensor_scalar_mul(out=msq[:, sl], in0=sq_ps[tch], scalar1=1.0 / C)
            musq = rowp.tile([1, T], F32, tag="row", name="musq")
            nc.vector.tensor_tensor(out=musq, in0=mu, in1=mu, op=OP.mult)
            nc.vector.tensor_tensor(out=msq, in0=msq, in1=musq, op=OP.subtract)
            nc.scalar.activation(out=musq, in_=msq, func=AF.Sqrt, bias=eps_t, scale=1.0)
            arow = rowp.tile([1, T], F32R, tag="row", name="arow")
            nc.vector.reciprocal(out=arow, in_=musq)
            brow = rowp.tile([1, T], F32R, tag="row", name="brow")
            nc.vector.scalar_tensor_tensor(out=brow, in0=mu, scalar=-1.0, in1=arow,
                                           op0=OP.mult, op1=OP.mult)
            A_b = abp.tile([P, T], F32, tag="A_b", name="A_b")
            B_b = abp.tile([P, T], F32, tag="B_b", name="B_b")
            for tch in range(2):
                sl = slice(512 * tch, 512 * (tch + 1))
                for row, dst in ((arow, A_b), (brow, B_b)):
                    bps = psp.tile([P, 512], F32, tag="bc", name="bc")
                    nc.tensor.matmul(bps, ones_1x128, row[:, sl], start=True, stop=True)
                    nc.scalar.copy(out=dst[:, sl], in_=bps)
            return A_b, B_b

        def ln_apply(xtiles, A_b, B_b, gcol, bcol, hpool, tsl=slice(0, T)):
            w = tsl.stop - tsl.start
            htiles = []
            for c in range(NCT):
                t1 = scrp.tile([P, T], F32, tag="lnscr", name="lnscr")
                nc.vector.scalar_tensor_tensor(
                    out=t1[:, 0:w], in0=xtiles[c][:, tsl], scalar=gcol[:, c:c + 1],
                    in1=A_b[:, tsl], op0=OP.mult, op1=OP.mult)
                nc.vector.scalar_tensor_tensor(
                    out=t1[:, 0:w], in0=B_b[:, tsl], scalar=gcol[:, c:c + 1],
                    in1=t1[:, 0:w], op0=OP.mult, op1=OP.add)
                ht = hpool.tile([P, w], F32R, tag="h", name="h")
                nc.scalar.activation(out=ht, in_=t1[:, 0:w], func=AF.Identity,
                                     bias=bcol[:, c:c + 1], scale=1.0)
                htiles.append(ht)
            return htiles

        def attn_chunk(kq_of, vaug_tiles, n_s, otiles, h, tch, psp, ppool, causal):
            (kt, ko), (qt, qo) = kq_of(h)
            tsl = slice(512 * tch, 512 * (tch + 1))
            ptiles = []
            sp_tiles = []
            for st in range(n_s):
                sps = psp.tile([P, 512], F32, tag="s", name="s")
                h0 = h - h % 2  # pack the head pair's S matmuls back to back so
                # their disjoint PE row groups (tile_position) run concurrently
                nc.tensor.matmul(sps, kt[ko:ko + D, st * P:(st + 1) * P],
                                 qt[qo:qo + D, tsl], start=True, stop=True,
                                 tile_position=(ko, 0))
                sp_tiles.append(sps)
            for st in range(n_s):
                sps = sp_tiles[st]
                pt = ppool.tile([P, 512], F32R, tag="p", name="p")
                j = st - 4 * tch
                if causal and j >= 0:
                    z = P * j
                    if z:
                        nc.vector.tensor_copy(out=pt[:, 0:z], in_=zeros384[:, 0:z])
                    nc.scalar.activation(out=pt[:, z:512], in_=sps[:, z:512],
                                         func=AF.Exp, scale=0.125)
                    nc.vector.tensor_tensor(out=pt[:, z:z + P], in0=pt[:, z:z + P],
                                            in1=master[:, 384:512], op=OP.mult)
                else:
                    nc.scalar.activation(out=pt, in_=sps, func=AF.Exp, scale=0.125)
                ptiles.append(pt)
            ops = psp.tile([65, 512], F32, tag="o", name="o")
            for st in range(n_s):
                nc.tensor.matmul(ops, vaug_tiles[st][:, 65 * h:65 * h + 65],
                                 ptiles[st], start=(st == 0), stop=(st == n_s - 1))
            rden = rbp.tile([1, 512], F32R, tag="rden", name="rden")
            nc.vector.reciprocal(out=rden, in_=ops[64:65, :])
            bps = psp.tile([64, 512], F32, tag="b", name="b")
            nc.tensor.matmul(bps, ones_1x128[:, 0:64], rden, start=True, stop=True)
            rb = rbp.tile([64, 512], F32, tag="rb", name="rb")
            nc.scalar.copy(out=rb, in_=bps)
            ot = otiles[h // 2]
            po = (h % 2) * D
            nc.vector.tensor_tensor(out=ot[po:po + D, tsl], in0=ops[0:64, :], in1=rb,
                                    op=OP.mult)

        # ================= P0: load & transpose x, LN1, qkv ==================
        xT_cm, xTp = openp(name="xT", bufs=NCT, side="right")
        h1_cm, hp = openp(name="h1", bufs=NCT)

        tok_cm, tokp = openp(name="tok0", bufs=2)
        tp_cm, tpp = openp(name="psT0", bufs=2, space="PSUM")
        xT = [xTp.tile([P, T], F32R, tag="xT", name="xT") for _ in range(NCT)]
        for tt in range(NTT):
            tok = tokp.tile([P, C], F32, tag="tok", name="tok")
            nc.sync.dma_start(out=tok, in_=dr["x"].ap()[tt * P:(tt + 1) * P, :])
            for c in range(NCT):
                tps = tpp.tile([P, P], F32, tag="tp", name="tp")
                nc.tensor.transpose(tps, tok[:, c * P:(c + 1) * P], ident)
                nc.vector.tensor_copy(out=xT[c][:, tt * P:(tt + 1) * P], in_=tps)
        tp_cm.__exit__(None, None, None)
        tok_cm.__exit__(None, None, None)

        ln_cm, lnp = openp(name="psLN0", bufs=2, space="PSUM")
        A_b, B_b = ln_stats(xT, lnp)
        ln_cm.__exit__(None, None, None)
        h_t = ln_apply(xT, A_b, B_b, g1, b1, hp)
        for c in range(NCT):
            nc.sync.dma_start(out=xT_d[c], in_=xT[c])
        xT_cm.__exit__(None, None, None)

        # v first (x-stationary), then q,k (W-stationary)
        vap_cm, vap = openp(name="vaug", bufs=NTT, side="right")
        vaug = [vap.tile([P, 16 * 65], F32R, tag="va", name="va") for _ in range(NTT)]

        wv_cm, wv = openp(name="wv", bufs=2)
        accv_cm, accv = openp(name="psACv", bufs=3, space="PSUM")
        brow_v = rowp.tile([1, C], F32, tag="row", name="braw")
        nc.sync.dma_start(out=brow_v,
                          in_=dr["b_attn"].ap()[2 * C:3 * C].rearrange("(a c) -> a c", a=1))
        bvb1 = bcast_row(brow_v, wv, accv, "bvb")
        for cc in range(4):   # v output chunks of 256 cols (4 heads each)
            stage = wv.tile([P, NCT, 256], F32, tag="vws", name="vws")
            nc.sync.dma_start(
                out=stage,
                in_=W2d("W_attn")[:, 2 * C + 256 * cc: 2 * C + 256 * (cc + 1)]
                .rearrange("(c p) f -> p c f", p=P))
            wr = wv.tile([P, NCT, 256], F32R, tag="vwr", name="vwr")
            nc.gpsimd.tensor_copy(out=wr, in_=stage)
            for tt in range(NTT):
                vps = accv.tile([P, 256], F32, tag="acc", name="acc")
                for c in range(NCT):
                    nc.tensor.matmul(vps, h_t[c][:, tt * P:(tt + 1) * P], wr[:, c, :],
                                     start=(c == 0), stop=(c == NCT - 1))
                dst = vaug[tt].rearrange("p (h x) -> p h x", x=65)[:, 4 * cc:4 * (cc + 1), 0:64]
                nc.vector.tensor_tensor(
                    out=dst, in0=vps.rearrange("p (h x) -> p h x", x=64),
                    in1=bvb1[:, 256 * cc:256 * (cc + 1)].rearrange("p (h x) -> p h x", x=64),
                    op=OP.add)
        for tt in range(NTT):
            nc.vector.tensor_copy(
                out=vaug[tt].rearrange("p (h x) -> p h x", x=65)[:, :, 64:65],
                in_=ones_col.rearrange("p (h x) -> p h x", x=1))
        accv_cm.__exit__(None, None, None)
        wv_cm.__exit__(None, None, None)

        qk_cm, qkp = openp(name="qk", bufs=16, side="right")
        w1_cm, w1 = openp(name="w1", bufs=2)
        acc_cm, accp = openp(name="psAC1", bufs=3, space="PSUM")
        qk_t = []
        for f in range(16):
            wsl = load_wslab(W2d("W_attn"), f, w1)
            qt = qkp.tile([P, T], F32R, tag="qk", name="qk")
            for tch in range(2):
                aps = accp.tile([P, 512], F32, tag="acc", name="acc")
                for c in range(NCT):
                    nc.tensor.matmul(aps, wsl[:, c, :], h_t[c][:, 512 * tch:512 * (tch + 1)],
                                     start=(c == 0), stop=(c == NCT - 1))
                nc.scalar.activation(out=qt[:, 512 * tch:512 * (tch + 1)], in_=aps,
                                     func=AF.Identity, bias=bqk[:, f:f + 1], scale=1.0)
            qk_t.append(qt)
        acc_cm.__exit__(None, None, None)
        w1_cm.__exit__(None, None, None)
        h1_cm.__exit__(None, None, None)

        # ================= P2: self attention =================
        o_cm, opool = openp(name="o1", bufs=NCT)
        pp_cm, pp = openp(name="pp1", bufs=8)
        psS_cm, psS = openp(name="psS1", bufs=2, space="PSUM")

        otiles = [opool.tile([P, T], F32R, tag="ot", name="ot") for _ in range(NCT)]

        def kq_self(h):
            return (qk_t[8 + h // 2], (h % 2) * D), (qk_t[h // 2], (h % 2) * D)

        for tch in range(2):
            for h in range(H):
                attn_chunk(kq_self, vaug, 4 * (tch + 1), otiles, h, tch, psS, pp,
                           causal=True)

        psS_cm.__exit__(None, None, None)
        pp_cm.__exit__(None, None, None)
        qk_cm.__exit__(None, None, None)
        vap_cm.__exit__(None, None, None)

        # ================= P3: aproj + residual (x1 = x + sa) =================
        res_cm, residp = openp(name="resid", bufs=NCT, side="right")
        resid = [residp.tile([P, T], F32R, tag="res", name="res") for _ in range(NCT)]

        xo_cm, xop = openp(name="xold", bufs=2)
        w2_cm, w2 = openp(name="w2", bufs=2)
        acc_cm, accp = openp(name="psAC3", bufs=3, space="PSUM")
        for co in range(NCT):
            wsl = load_wslab(W2d("W_aproj"), co, w2)
            xold = xop.tile([P, T], F32R, tag="xold", name="xold")
            nc.sync.dma_start(out=xold, in_=xT_d[co])
            for tch in range(2):
                sl = slice(512 * tch, 512 * (tch + 1))
                aps = accp.tile([P, 512], F32, tag="acc", name="acc")
                for c in range(NCT):
                    nc.tensor.matmul(aps, wsl[:, c, :], otiles[c][:, sl],
                                     start=(c == 0), stop=(c == NCT - 1))
                nc.vector.scalar_tensor_tensor(
                    out=resid[co][:, sl], in0=aps, scalar=bap_c[:, co:co + 1],
                    in1=xold[:, sl], op0=OP.add, op1=OP.add)
        acc_cm.__exit__(None, None, None)
        w2_cm.__exit__(None, None, None)
        xo_cm.__exit__(None, None, None)
        o_cm.__exit__(None, None, None)

        # ================= P4: cross attention =================
        ln_cm, lnp = openp(name="psLN1", bufs=2, space="PSUM")
        A_b, B_b = ln_stats(resid, lnp)
        ln_cm.__exit__(None, None, None)

        # right-stack: k2, v2, then q2 (live until end of cross attention)
        k2_cm, k2p = openp(name="k2", bufs=NCT, side="right")
        v2_cm, v2p = openp(name="v2", bufs=2, side="right")

        w3_cm, w3 = openp(name="w3", bufs=2)

        img_cm, imgp = openp(name="img", bufs=NCT)
        tok_cm, tokp = openp(name="tok4", bufs=2)
        tp_cm, tpp = openp(name="psT4", bufs=2, space="PSUM")
        imgT = [imgp.tile([P, TI], F32R, tag="imgT", name="imgT") for _ in range(NCT)]
        for tt in range(TI // P):
            tok = tokp.tile([P, C], F32, tag="tok", name="tok")
            nc.sync.dma_start(out=tok, in_=dr["x_img_feats"].ap()[tt * P:(tt + 1) * P, :])
            for c in range(NCT):
                tps = tpp.tile([P, P], F32, tag="tp", name="tp")
                nc.tensor.transpose(tps, tok[:, c * P:(c + 1) * P], ident)
                nc.vector.tensor_copy(out=imgT[c][:, tt * P:(tt + 1) * P], in_=tps)
        tp_cm.__exit__(None, None, None)
        tok_cm.__exit__(None, None, None)

        acc_cm, accp = openp(name="psAC4", bufs=2, space="PSUM")
        k2_t = []
        for f in range(NCT):
            wsl = load_wslab(W2d("Wk"), f, w3, eng=(nc.vector if f % 2 else nc.gpsimd))
            kt = k2p.tile([P, TI], F32R, tag="k2", name="k2")
            aps = accp.tile([P, 512], F32, tag="acc", name="acc")
            for c in range(NCT):
                nc.tensor.matmul(aps[:, 0:TI], wsl[:, c, :], imgT[c],
                                 start=(c == 0), stop=(c == NCT - 1))
            nc.scalar.activation(out=kt, in_=aps[:, 0:TI], func=AF.Identity,
                                 bias=bk_c[:, f:f + 1], scale=1.0)
            k2_t.append(kt)

        brow_v2 = rowp.tile([1, C], F32, tag="row", name="braw2")
        nc.sync.dma_start(out=brow_v2, in_=dr["bv"].ap().rearrange("(a c) -> a c", a=1))
        wv2_cm, wv2 = openp(name="wv2", bufs=2)
        bvb2 = bcast_row(brow_v2, wv2, accp, "bvb2")

        v2aug = [v2p.tile([P, 16 * 65], F32R, tag="va2", name="va2")
                 for _ in range(TI // P)]
        for cc in range(4):
            stage = wv2.tile([P, NCT, 256], F32, tag="vws", name="vws")
            nc.sync.dma_start(
                out=stage,
                in_=W2d("Wv")[:, 256 * cc: 256 * (cc + 1)].rearrange("(c p) f -> p c f", p=P))
            wr = wv2.tile([P, NCT, 256], F32R, tag="vwr", name="vwr")
            nc.gpsimd.tensor_copy(out=wr, in_=stage)
            for st in range(TI // P):
                vps = accp.tile([P, 256], F32, tag="acc2", name="acc2")
                for c in range(NCT):
                    nc.tensor.matmul(vps, imgT[c][:, st * P:(st + 1) * P], wr[:, c, :],
                                     start=(c == 0), stop=(c == NCT - 1))
                dst = v2aug[st].rearrange("p (h x) -> p h x", x=65)[:, 4 * cc:4 * (cc + 1), 0:64]
                nc.vector.tensor_tensor(
                    out=dst, in0=vps.rearrange("p (h x) -> p h x", x=64),
                    in1=bvb2[:, 256 * cc:256 * (cc + 1)].rearrange("p (h x) -> p h x", x=64),
                    op=OP.add)
        for st in range(TI // P):
            nc.vector.tensor_copy(
                out=v2aug[st].rearrange("p (h x) -> p h x", x=65)[:, :, 64:65],
                in_=ones_col.rearrange("p (h x) -> p h x", x=1))
        wv2_cm.__exit__(None, None, None)
        img_cm.__exit__(None, None, None)

        q2_cm, q2p = openp(name="q2", bufs=NCT, side="right")
        hp_cm, hp = openp(name="h2", bufs=NCT)
        hb_t = ln_apply(resid, A_b, B_b, g1, b1, hp)
        q2_t = []
        for f in range(NCT):
            wsl = load_wslab(W2d("Wq"), f, w3)
            qt = q2p.tile([P, T], F32R, tag="q2", name="q2")
            for tch in range(2):
                aps = accp.tile([P, 512], F32, tag="acc", name="acc")
                for c in range(NCT):
                    nc.tensor.matmul(aps, wsl[:, c, :], hb_t[c][:, 512 * tch:512 * (tch + 1)],
                                     start=(c == 0), stop=(c == NCT - 1))
                nc.scalar.activation(out=qt[:, 512 * tch:512 * (tch + 1)], in_=aps,
                                     func=AF.Identity, bias=bq_c[:, f:f + 1], scale=1.0)
            q2_t.append(qt)
        hp_cm.__exit__(None, None, None)
        acc_cm.__exit__(None, None, None)
        w3_cm.__exit__(None, None, None)

        o_cm, opool = openp(name="o2", bufs=NCT)
        pp_cm, pp = openp(name="pp2", bufs=6)
        psS_cm, psS = openp(name="psS2", bufs=2, space="PSUM")

        o2tiles = [opool.tile([P, T], F32R, tag="ot", name="ot") for _ in range(NCT)]

        def kq_cross(h):
            return (k2_t[h // 2], (h % 2) * D), (q2_t[h // 2], (h % 2) * D)

        for tch in range(2):
            for h in range(H):
                attn_chunk(kq_cross, v2aug, TI // P, o2tiles, h, tch, psS, pp,
                           causal=False)

        psS_cm.__exit__(None, None, None)
        pp_cm.__exit__(None, None, None)
        q2_cm.__exit__(None, None, None)
        v2_cm.__exit__(None, None, None)
        k2_cm.__exit__(None, None, None)

        # ================= P5: cproj + residual (x2, in place) =================
        w4_cm, w4 = openp(name="w4", bufs=2)
        acc_cm, accp = openp(name="psAC5", bufs=3, space="PSUM")
        for co in range(NCT):
            wsl = load_wslab(W2d("Wcproj"), co, w4)
            for tch in range(2):
                sl = slice(512 * tch, 512 * (tch + 1))
                aps = accp.tile([P, 512], F32, tag="acc", name="acc")
                for c in range(NCT):
                    nc.tensor.matmul(aps, wsl[:, c, :], o2tiles[c][:, sl],
                                     start=(c == 0), stop=(c == NCT - 1))
                nc.vector.scalar_tensor_tensor(
                    out=resid[co][:, sl], in0=aps, scalar=bcp_c[:, co:co + 1],
                    in1=resid[co][:, sl], op0=OP.add, op1=OP.add)
        acc_cm.__exit__(None, None, None)
        w4_cm.__exit__(None, None, None)
        o_cm.__exit__(None, None, None)

        # ================= P6: MLP =================
        ln_cm, lnp = openp(name="psLN2", bufs=2, space="PSUM")
        A_b, B_b = ln_stats(resid, lnp)
        ln_cm.__exit__(None, None, None)

        up_cm, up = openp(name="u", bufs=16, side="right")

        for tch in range(2):
            tsl = slice(512 * tch, 512 * (tch + 1))
            hp_cm, hp = openp(name=f"h3{tch}", bufs=NCT)
            h2_t = ln_apply(resid, A_b, B_b, g2, b2, hp, tsl=tsl)
            utiles = [up.tile([P, 2, 512], F32R, tag="u", name="u") for _ in range(16)]
            w5_cm, w5 = openp(name=f"w5{tch}", bufs=3)
            acc_cm, accp = openp(name=f"psU{tch}", bufs=2, space="PSUM")
            for ff in range(NFT):
                wsl = load_wslab(W2d("W_fc"), ff, w5,
                                 eng=(nc.vector if ff % 2 else nc.gpsimd))
                ups = accp.tile([P, 512], F32, tag="acc", name="acc")
                for c in range(NCT):
                    nc.tensor.matmul(ups, wsl[:, c, :], h2_t[c],
                                     start=(c == 0), stop=(c == NCT - 1))
                nc.scalar.activation(out=utiles[ff // 2][:, ff % 2, :], in_=ups,
                                     func=AF.Gelu_apprx_tanh,
                                     bias=bfc_c[:, ff:ff + 1], scale=1.0)
            acc_cm.__exit__(None, None, None)
            w5_cm.__exit__(None, None, None)
            hp_cm.__exit__(None, None, None)

            w6_cm, w6 = openp(name=f"w6{tch}", bufs=4)
            psM_cm, psM = openp(name=f"psM{tch}", bufs=8, space="PSUM")
            mps = [psM.tile([P, 512], F32, tag="m", name="m") for _ in range(NCT)]
            for ff in range(NFT):
                stage = w6.tile([P, C], F32, tag="mps", name="mps")
                nc.sync.dma_start(out=stage, in_=W2d("W_mproj")[ff * P:(ff + 1) * P, :])
                wr = w6.tile([P, C], F32R, tag="mpr", name="mpr")
                (nc.vector if ff % 2 else nc.gpsimd).tensor_copy(out=wr, in_=stage)
                for co in range(NCT):
                    nc.tensor.matmul(mps[co], wr[:, co * P:(co + 1) * P],
                                     utiles[ff // 2][:, ff % 2, :],
                                     start=(ff == 0), stop=(ff == NFT - 1))
            for co in range(NCT):
                nc.vector.scalar_tensor_tensor(
                    out=resid[co][:, tsl], in0=mps[co], scalar=bmp_c[:, co:co + 1],
                    in1=resid[co][:, tsl], op0=OP.add, op1=OP.add)
            psM_cm.__exit__(None, None, None)
            w6_cm.__exit__(None, None, None)

        up_cm.__exit__(None, None, None)

        # ================= P7: transpose back & store =================
        tok_cm, tokp = openp(name="tok7", bufs=2)
        tp_cm, tpp = openp(name="psT7", bufs=4, space="PSUM")
        for tt in range(NTT):
            otok = tokp.tile([P, C], F32, tag="tok", name="tok")
            for c in range(NCT):
                tps = tpp.tile([P, P], F32R, tag="tpr", name="tpr")
                nc.tensor.transpose(tps, resid[c][:, tt * P:(tt + 1) * P], identR)
                nc.vector.tensor_copy(out=otok[:, c * P:(c + 1) * P], in_=tps)
            nc.sync.dma_start(out=out_d.ap()[tt * P:(tt + 1) * P, :], in_=otok)
        tp_cm.__exit__(None, None, None)
        tok_cm.__exit__(None, None, None)
        res_cm.__exit__(None, None, None)

        for cm in reversed(kw_cms):
            cm.__exit__(None, None, None)

    nc.compile()
    return nc


def kernel(**inputs):
    from concourse.bass_utils import run_bass_kernel_spmd

    if "nc" not in _CACHED:
        _CACHED["nc"] = _build()
    nc = _CACHED["nc"]

    np_inputs = {k: np.asarray(v, dtype=np.float32) for k, v in inputs.items()}
    in_maps = []
    for b in range(B):
        m = dict(np_inputs)
        m["x"] = np.ascontiguousarray(np_inputs["x"][b])
        m["x_img_feats"] = np.ascontiguousarray(np_inputs["x_img_feats"][b])
        in_maps.append(m)
    res = run_bass_kernel_spmd(nc, in_maps, core_ids=list(range(B)))
    out = np.stack([res.results[b]["out"] for b in range(B)], axis=0)
    return out.astype(np.float32)



# revision 7
# speedup vs baseline: 1.1487x; 1.1487x over previous
"""Trainium2 Bass kernel for a dense transformer block (self-attn + cross-attn + MLP).

Sharding: data-parallel over batch, one batch element per NeuronCore (B=8, 8 cores),
no collectives. Activations are feature-major ([C, T]) on chip.

All eight projection GEMMs run as fp8e4m3 DoubleRow matmuls (K=256 per
instruction). Weights are scaled x128, quantized and DR-interleaved on the HOST
in numpy, DMA'd straight into SBUF fp8 tiles. Error-sensitive GEMMs (v, aproj,
fc, mproj) use a 3-pass scheme at a single PSUM scale:
    psum = W8*h8 + W8*dh8 + R8*h8
where dh8 = fp8(h - h8) (subnormal range => ~0.1% effective activation error)
and R8 = fp8(128*(W - W8/128)) (weight residual). This gives ~bf16 accuracy at
0.75x of bf16's PE cost. Insensitive GEMMs (q,k, cross q2/k2/v2, cproj) run
1-pass fp8. q/k score operands are bf16; S/AV stay bf16/f32r.

The 1/128 weight scale is folded into every PSUM evacuation. The residual
stream lives in SBUF the whole kernel (no DRAM spill). Softmax exp runs on
paired PSUM banks ([128,1024] per ACT op); masking/squares/deltas run on
GPSIMD to keep DVE off the critical path.
"""

import sys
import numpy as np

sys.path.insert(0, "/opt/trn_rl_repo")

B, T, C = 8, 1024, 1024
H = 16
D = C // H          # 64
TI = 256
FF = 4 * C          # 4096
EPS = 1e-5
NCT = C // 128      # 8 c tiles
NTT = T // 128      # 8 t tiles
P = 128
WS = 128.0          # fp8 weight scale
WSI = 1.0 / WS

_CACHED = {}

# packed fp8 weights: [MC, 128, KK*2*Mc], elem [mc, p, (kk, ko, m)] =
# q8(W*WS)[256*kk + 128*ko + p, mc*Mc + m]; *R_p carry the fp8 residual
# R = WS*W - deq(W8) in the same layout.
WPACK = {
    "attn_p": (C, 3 * C, 256),
    "vR_p": (C, C, 256),
    "aproj_p": (C, C, 256),
    "aprojR_p": (C, C, 256),
    "q_p": (C, C, 256),
    "k_p": (C, C, 256),
    "v2_p": (C, C, 256),
    "cproj_p": (C, C, 256),
    "fc_p": (C, FF, 256),
    "fcR_p": (C, FF, 256),
    "mproj_p": (FF, C, 256),
    "mprojR_p": (FF, C, 256),
}


def _build(flags):
    import concourse.tile as tile
    from concourse import bacc, mybir
    from concourse.masks import make_identity

    F32, F32R = mybir.dt.float32, mybir.dt.float32r
    BF16 = mybir.dt.bfloat16
    F8 = mybir.dt.float8e4
    AF = mybir.ActivationFunctionType
    OP = mybir.AluOpType
    DR = mybir.MatmulPerfMode.DoubleRow

    ab_bias, cp_bias, mp_bias, ln1z, ln2z = flags

    nc = bacc.Bacc("TRN2", target_bir_lowering=False, debug=False, num_devices=8)

    dr = {}
    dr["x"] = nc.dram_tensor("x", [T, C], F32, kind="ExternalInput")
    dr["x_img_feats"] = nc.dram_tensor("x_img_feats", [TI, C], F32, kind="ExternalInput")
    for nm, shp in [
        ("ln1_g", [C]), ("ln1_b", [C]), ("ln2_g", [C]), ("ln2_b", [C]),
        ("b_attn", [3 * C]), ("b_aproj", [C]),
        ("bq", [C]), ("bk", [C]), ("bv", [C]), ("bcproj", [C]),
        ("b_fc", [FF]), ("b_mproj", [C]),
    ]:
        dr[nm] = nc.dram_tensor(nm, shp, F32, kind="ExternalInput")
    for nm, (K, M, Mc) in WPACK.items():
        dr[nm] = nc.dram_tensor(nm, [M // Mc, P, (K // 256) * 2 * Mc], F8,
                                kind="ExternalInput")
    out_d = nc.dram_tensor("out", [T, C], F32, kind="ExternalOutput")

    with tile.TileContext(nc) as tc, nc.allow_low_precision(
        reason="fp8 DoubleRow projections + bf16 attention are intentional"
    ):
        kw_cms = []

        def openp(**kw):
            cm = tc.tile_pool(**kw)
            return cm, cm.__enter__()

        def openkw(**kw):
            cm, p = openp(**kw)
            kw_cms.append(cm)
            return p

        # ---------------- kernel-wide pools (left-stack base) ----------------
        constp = openkw(name="const", bufs=1)
        scrp = openkw(name="scr", bufs=2)       # f32 [128,1024] ln scratch
        fsrp = openkw(name="fsr", bufs=2)       # f32r [128,512] squares
        abp = openkw(name="ab", bufs=1)         # A_b/B_b [128,1024]
        rowp = openkw(name="rows", bufs=5)      # one [1,1024] "row" tag
        rbp = openkw(name="rb", bufs=3)         # [64,512] + [1,512] rden
        osc = openkw(name="osc", bufs=4)        # [64,512] o-split scratch

        # ---------------- constants ----------------
        ident = constp.tile([P, P], F32)
        make_identity(nc, ident)
        identR = constp.tile([P, P], F32R)
        nc.vector.tensor_copy(out=identR, in_=ident)

        ones_col = constp.tile([P, 16], F32)
        nc.vector.memset(ones_col, 1.0)
        ones128R = constp.tile([P, 1], F32R)
        nc.vector.tensor_copy(out=ones128R, in_=ones_col[:, 0:1])
        o1x = constp.tile([1, P], F32)
        nc.vector.memset(o1x, 1.0)
        ones_1x128 = constp.tile([1, P], F32R)
        nc.vector.tensor_copy(out=ones_1x128, in_=o1x)
        eps_t = constp.tile([1, 1], F32)
        nc.vector.memset(eps_t, EPS)
        zeros384 = constp.tile([P, 384], F32)
        nc.vector.memset(zeros384, 0.0)

        # master causal mask [128, 896]: keep (1.0) iff (col - row - 384) >= 0.
        master = constp.tile([P, 896], F32)
        nc.gpsimd.memset(master, 1.0)
        nc.gpsimd.affine_select(
            out=master, in_=master, compare_op=OP.is_ge, fill=0.0,
            base=-384, pattern=[[1, 896]], channel_multiplier=-1)

        # ================= P0: load & transpose x (issued first) =============
        res_cm, residp = openp(name="resid", bufs=NCT, side="right")
        resid = [residp.tile([P, T], F32R, tag="res", name="res") for _ in range(NCT)]

        tok_cm, tokp = openp(name="tok0", bufs=2)
        tp_cm, tpp = openp(name="psT0", bufs=2, space="PSUM")
        for tt in range(NTT):
            tok = tokp.tile([P, C], F32, tag="tok", name="tok")
            nc.sync.dma_start(out=tok, in_=dr["x"].ap()[tt * P:(tt + 1) * P, :])
            for c in range(NCT):
                tps = tpp.tile([P, P], F32, tag="tp", name="tp")
                nc.tensor.transpose(tps, tok[:, c * P:(c + 1) * P], ident)
                if c % 2:
                    nc.vector.tensor_copy(out=resid[c][:, tt * P:(tt + 1) * P], in_=tps)
                else:
                    nc.scalar.copy(out=resid[c][:, tt * P:(tt + 1) * P], in_=tps)
        tp_cm.__exit__(None, None, None)
        tok_cm.__exit__(None, None, None)

        # ---------------- small input rows (issued after x) ----------------
        def load_cols(name, nf):
            t = constp.tile([P, nf], F32, name=name + "_c")
            nc.sync.dma_start(out=t, in_=dr[name].ap().rearrange("(f p) -> p f", p=P))
            return t

        g1, b1 = load_cols("ln1_g", NCT), load_cols("ln1_b", NCT)
        g2, b2 = load_cols("ln2_g", NCT), load_cols("ln2_b", NCT)
        bqk = constp.tile([P, 16], F32)
        nc.sync.dma_start(out=bqk, in_=dr["b_attn"].ap()[0:2 * C].rearrange("(f p) -> p f", p=P))
        bq_c = load_cols("bq", NCT)
        bk_c = load_cols("bk", NCT)
        bap_c = load_cols("b_aproj", NCT)
        bcp_c = load_cols("bcproj", NCT)
        bmp_c = load_cols("b_mproj", NCT)
        bfc_c = load_cols("b_fc", FF // 128)

        # ---------------- helpers ----------------
        def load_wp(name, mc, wpool):
            K, M, Mc = WPACK[name]
            KK = K // 256
            t = wpool.tile([P, KK, 2, Mc], F8, tag="wp", name="wp")
            nc.sync.dma_start(
                out=t,
                in_=dr[name].ap()[mc].rearrange("p (kk ko m) -> p kk ko m", kk=KK, ko=2))
            return t

        def bcast_row(row_f32, dest_pool, psp, tag):
            rowr = rowp.tile([1, C], F32R, tag="row", name="rowr")
            nc.vector.tensor_copy(out=rowr, in_=row_f32)
            dest = dest_pool.tile([P, C], F32, tag=tag, name=tag)
            for cc in range(2):
                bps = psp.tile([P, 512], F32, tag="bc", name="bc")
                nc.tensor.matmul(bps, ones_1x128, rowr[:, 512 * cc:512 * (cc + 1)],
                                 start=True, stop=True)
                nc.scalar.copy(out=dest[:, 512 * cc:512 * (cc + 1)], in_=bps)
            return dest

        def ln_stats(xtiles, psp):
            sum_ps, sq_ps = [], []
            for tch in range(2):
                sp = psp.tile([1, 512], F32, tag="lnsum", name="lnsum")
                qp = psp.tile([1, 512], F32, tag="lnsq", name="lnsq")
                for c in range(NCT):
                    xs = xtiles[c][:, 512 * tch:512 * (tch + 1)]
                    nc.tensor.matmul(sp, ones128R, xs, start=(c == 0), stop=(c == NCT - 1))
                    sq = fsrp.tile([P, 512], F32R, tag="sq", name="sq")
                    nc.gpsimd.tensor_tensor(out=sq, in0=xs, in1=xs, op=OP.mult)
                    nc.tensor.matmul(qp, ones128R, sq, start=(c == 0), stop=(c == NCT - 1))
                sum_ps.append(sp)
                sq_ps.append(qp)
            mu = rowp.tile([1, T], F32, tag="row", name="mu")
            msq = rowp.tile([1, T], F32, tag="row", name="msq")
            for tch in range(2):
                sl = slice(512 * tch, 512 * (tch + 1))
                nc.vector.tensor_scalar_mul(out=mu[:, sl], in0=sum_ps[tch], scalar1=1.0 / C)
                nc.vector.tensor_scalar_mul(out=msq[:, sl], in0=sq_ps[tch], scalar1=1.0 / C)
            musq = rowp.tile([1, T], F32, tag="row", name="musq")
            nc.vector.tensor_tensor(out=musq, in0=mu, in1=mu, op=OP.mult)
            nc.vector.tensor_tensor(out=msq, in0=msq, in1=musq, op=OP.subtract)
            nc.scalar.activation(out=musq, in_=msq, func=AF.Sqrt, bias=eps_t, scale=1.0)
            arow = rowp.tile([1, T], F32R, tag="row", name="arow")
            nc.vector.reciprocal(out=arow, in_=musq)
            brow = rowp.tile([1, T], F32R, tag="row", name="brow")
            nc.vector.scalar_tensor_tensor(out=brow, in0=mu, scalar=-1.0, in1=arow,
                                           op0=OP.mult, op1=OP.mult)
            A_b = abp.tile([P, T], F32, tag="A_b", name="A_b")
            B_b = abp.tile([P, T], F32, tag="B_b", name="B_b")
            for tch in range(2):
                sl = slice(512 * tch, 512 * (tch + 1))
                for row, dst in ((arow, A_b), (brow, B_b)):
                    bps = psp.tile([P, 512], F32, tag="bc", name="bc")
                    nc.tensor.matmul(bps, ones_1x128, row[:, sl], start=True, stop=True)
                    nc.scalar.copy(out=dst[:, sl], in_=bps)
            return A_b, B_b

        def ln_apply_f8(xtiles, A_b, B_b, gcol, bcol, h_all, d_all, lnz):
            """LN apply -> fp8 h_all[:, c, :]; optional fp8 delta d_all."""
            for c in range(NCT):
                t1 = scrp.tile([P, T], F32, tag="lnscr", name="lnscr")
                nc.vector.scalar_tensor_tensor(
                    out=t1, in0=xtiles[c], scalar=gcol[:, c:c + 1],
                    in1=A_b, op0=OP.mult, op1=OP.mult)
                if d_all is None and lnz:
                    nc.vector.scalar_tensor_tensor(
                        out=h_all[:, c, :], in0=B_b, scalar=gcol[:, c:c + 1],
                        in1=t1, op0=OP.mult, op1=OP.add)
                    continue
                nc.vector.scalar_tensor_tensor(
                    out=t1, in0=B_b, scalar=gcol[:, c:c + 1],
                    in1=t1, op0=OP.mult, op1=OP.add)
                if not lnz:
                    nc.scalar.activation(out=t1, in_=t1, func=AF.Identity,
                                         bias=bcol[:, c:c + 1], scale=1.0)
                nc.scalar.activation(out=h_all[:, c, :], in_=t1, func=AF.Identity,
                                     bias=0.0, scale=1.0)
                if d_all is not None:
                    nc.gpsimd.scalar_tensor_tensor(
                        out=d_all[:, c, :], in0=h_all[:, c, :], scalar=-1.0,
                        in1=t1, op0=OP.mult, op1=OP.add)

        def attn_chunk(kq_of, vaug_tiles, n_s, h, tch, psp, ppool, causal,
                       o_all, oD_all):
            (kt, ko), (qt, qo) = kq_of(h)
            tsl = slice(512 * tch, 512 * (tch + 1))
            ptiles = []
            pair_ps = []
            for pr in range(n_s // 2):
                sps = psp.tile([P, 1024], F32, tag="s", name="s")
                for hf in range(2):
                    st = 2 * pr + hf
                    nc.tensor.matmul(sps[:, 512 * hf:512 * hf + 512],
                                     kt[ko:ko + D, st * P:(st + 1) * P],
                                     qt[qo:qo + D, tsl], start=True, stop=True,
                                     tile_position=(ko, 0))
                pair_ps.append(sps)
            for pr in range(n_s // 2):
                sps = pair_ps[pr]
                pt = ppool.tile([P, 1024], F32R, tag="p", name="p")
                j0 = 2 * pr - 4 * tch
                j1 = j0 + 1
                d0 = causal and j0 >= 0
                d1 = causal and j1 >= 0
                z0 = P * j0 if d0 else 0
                z1 = P * j1 if d1 else 0
                nc.scalar.activation(out=pt[:, z0:1024], in_=sps[:, z0:1024],
                                     func=AF.Exp, scale=0.125)
                if d0 and z0:
                    nc.gpsimd.tensor_copy(out=pt[:, 0:z0], in_=zeros384[:, 0:z0])
                if d1 and z1:
                    nc.gpsimd.tensor_copy(out=pt[:, 512:512 + z1], in_=zeros384[:, 0:z1])
                if d0:
                    nc.gpsimd.tensor_tensor(out=pt[:, z0:z0 + P], in0=pt[:, z0:z0 + P],
                                            in1=master[:, 384:512], op=OP.mult)
                if d1:
                    nc.gpsimd.tensor_tensor(out=pt[:, 512 + z1:512 + z1 + P],
                                            in0=pt[:, 512 + z1:512 + z1 + P],
                                            in1=master[:, 384:512], op=OP.mult)
                ptiles.append(pt)
            ops = psp.tile([65, 512], F32, tag="o", name="o")
            for st in range(n_s):
                pt = ptiles[st // 2][:, 512 * (st % 2):512 * (st % 2) + 512]
                nc.tensor.matmul(ops, vaug_tiles[st][:, 65 * h:65 * h + 65],
                                 pt, start=(st == 0), stop=(st == n_s - 1))
            rden = rbp.tile([1, 512], F32R, tag="rden", name="rden")
            nc.vector.reciprocal(out=rden, in_=ops[64:65, :])
            bps = psp.tile([64, 512], F32, tag="b", name="b")
            nc.tensor.matmul(bps, ones_1x128[:, 0:64], rden, start=True, stop=True)
            rb = rbp.tile([64, 512], F32, tag="rb", name="rb")
            nc.scalar.copy(out=rb, in_=bps)
            po = (h % 2) * D
            c = h // 2
            if oD_all is None:
                nc.vector.tensor_tensor(out=o_all[po:po + D, c, tsl],
                                        in0=ops[0:64, :], in1=rb, op=OP.mult)
            else:
                oscr = osc.tile([64, 512], F32R, tag="oscr", name="oscr")
                nc.vector.tensor_tensor(out=oscr, in0=ops[0:64, :], in1=rb, op=OP.mult)
                nc.gpsimd.tensor_copy(out=o_all[po:po + D, c, tsl], in_=oscr)
                nc.gpsimd.scalar_tensor_tensor(
                    out=oD_all[po:po + D, c, tsl], in0=o_all[po:po + D, c, tsl],
                    scalar=-1.0, in1=oscr, op0=OP.mult, op1=OP.add)

        def dr_group(psum, pairs):
            n = len(pairs)
            for i, (lh, rh) in enumerate(pairs):
                nc.tensor.matmul(psum, lh, rh, start=(i == 0), stop=(i == n - 1),
                                 perf_mode=DR)

        def ws_passes(wt, wtR, h8, hd, msl, tsl2):
            ps = [(wt[:, kk, :, msl], h8[:, 2 * kk:2 * kk + 2, tsl2]) for kk in range(4)]
            if hd is not None:
                ps += [(wt[:, kk, :, msl], hd[:, 2 * kk:2 * kk + 2, tsl2]) for kk in range(4)]
            if wtR is not None:
                ps += [(wtR[:, kk, :, msl], h8[:, 2 * kk:2 * kk + 2, tsl2]) for kk in range(4)]
            return ps

        # ================= P1: LN1 + qkv projections =================
        ln_cm, lnp = openp(name="psLN0", bufs=2, space="PSUM")
        A_b, B_b = ln_stats(resid, lnp)
        ln_cm.__exit__(None, None, None)

        h1_cm, hp = openp(name="h1", bufs=1)
        h1 = hp.tile([P, NCT, T], F8, tag="h", name="h")
        hd1 = hp.tile([P, NCT, T], F8, tag="hd", name="hd")
        ln_apply_f8(resid, A_b, B_b, g1, b1, h1, hd1, ln1z)

        # v first (h-stationary -> token-major v), then q,k (W-stationary)
        vap_cm, vap = openp(name="vaug", bufs=NTT, side="right")
        vaug = [vap.tile([P, 16 * 65], F32R, tag="va", name="va") for _ in range(NTT)]

        wv_cm, wv = openp(name="wv", bufs=4)
        accv_cm, accv = openp(name="psACv", bufs=3, space="PSUM")
        brow_v = rowp.tile([1, C], F32, tag="row", name="braw")
        nc.sync.dma_start(out=brow_v,
                          in_=dr["b_attn"].ap()[2 * C:3 * C].rearrange("(a c) -> a c", a=1))
        bvb1 = bcast_row(brow_v, wv, accv, "bvb")
        for cc in range(4):   # v output chunks of 256 cols (4 heads each)
            wt = load_wp("attn_p", 8 + cc, wv)
            wtR = load_wp("vR_p", cc, wv)
            for tt in range(NTT):
                vps = accv.tile([P, 256], F32, tag="acc", name="acc")
                tsl = slice(tt * P, (tt + 1) * P)
                ps = ([(h1[:, 2 * kk:2 * kk + 2, tsl], wt[:, kk, :, :]) for kk in range(4)]
                      + [(hd1[:, 2 * kk:2 * kk + 2, tsl], wt[:, kk, :, :]) for kk in range(4)]
                      + [(h1[:, 2 * kk:2 * kk + 2, tsl], wtR[:, kk, :, :]) for kk in range(4)])
                dr_group(vps, ps)
                dst = vaug[tt].rearrange("p (h x) -> p h x", x=65)[:, 4 * cc:4 * (cc + 1), 0:64]
                nc.vector.scalar_tensor_tensor(
                    out=dst, in0=vps.rearrange("p (h x) -> p h x", x=64),
                    scalar=WSI,
                    in1=bvb1[:, 256 * cc:256 * (cc + 1)].rearrange("p (h x) -> p h x", x=64),
                    op0=OP.mult, op1=OP.add)
        for tt in range(NTT):
            nc.vector.tensor_copy(
                out=vaug[tt].rearrange("p (h x) -> p h x", x=65)[:, :, 64:65],
                in_=ones_col.rearrange("p (h x) -> p h x", x=1))
        accv_cm.__exit__(None, None, None)
        wv_cm.__exit__(None, None, None)

        qk_cm, qkp = openp(name="qk", bufs=16, side="right")
        w1_cm, w1 = openp(name="w1", bufs=3)
        acc_cm, accp = openp(name="psAC1", bufs=3, space="PSUM")
        qk_t = []
        for mc in range(8):
            wt = load_wp("attn_p", mc, w1)
            for mh in range(2):
                f = 2 * mc + mh
                qt = qkp.tile([P, T], BF16, tag="qk", name="qk")
                for tch in range(2):
                    aps = accp.tile([P, 512], F32, tag="acc", name="acc")
                    dr_group(aps, ws_passes(wt, None, h1, None,
                                            slice(128 * mh, 128 * mh + 128),
                                            slice(512 * tch, 512 * (tch + 1))))
                    nc.scalar.activation(out=qt[:, 512 * tch:512 * (tch + 1)], in_=aps,
                                         func=AF.Identity, bias=bqk[:, f:f + 1], scale=WSI)
                qk_t.append(qt)
        acc_cm.__exit__(None, None, None)
        w1_cm.__exit__(None, None, None)
        h1_cm.__exit__(None, None, None)

        # ================= P2: self attention =================
        o_cm, opool = openp(name="o1", bufs=1)
        o_all = opool.tile([P, NCT, T], F8, tag="ot", name="ot")
        oD_all = opool.tile([P, NCT, T], F8, tag="otd", name="otd")
        pp_cm, pp = openp(name="pp1", bufs=5)
        psS_cm, psS = openp(name="psS1", bufs=2, space="PSUM")

        def kq_self(h):
            return (qk_t[8 + h // 2], (h % 2) * D), (qk_t[h // 2], (h % 2) * D)

        for tch in range(2):
            for h in range(H):
                attn_chunk(kq_self, vaug, 4 * (tch + 1), h, tch, psS, pp,
                           causal=True, o_all=o_all, oD_all=oD_all)

        psS_cm.__exit__(None, None, None)
        pp_cm.__exit__(None, None, None)
        qk_cm.__exit__(None, None, None)
        vap_cm.__exit__(None, None, None)

        # ================= P3: aproj + residual (x1 = x + sa, in place) ======
        w2_cm, w2 = openp(name="w2", bufs=4)
        acc_cm, accp = openp(name="psAC3", bufs=3, space="PSUM")
        for mc in range(4):
            wt = load_wp("aproj_p", mc, w2)
            wtR = load_wp("aprojR_p", mc, w2)
            for mh in range(2):
                co = 2 * mc + mh
                for tch in range(2):
                    sl = slice(512 * tch, 512 * (tch + 1))
                    aps = accp.tile([P, 512], F32, tag="acc", name="acc")
                    dr_group(aps, ws_passes(wt, wtR, o_all, oD_all,
                                            slice(128 * mh, 128 * mh + 128), sl))
                    nc.vector.scalar_tensor_tensor(
                        out=resid[co][:, sl], in0=aps, scalar=WSI,
                        in1=resid[co][:, sl], op0=OP.mult, op1=OP.add)
                    if ab_bias:
                        nc.vector.tensor_scalar_add(
                            out=resid[co][:, sl], in0=resid[co][:, sl],
                            scalar1=bap_c[:, co:co + 1])
        acc_cm.__exit__(None, None, None)
        w2_cm.__exit__(None, None, None)
        o_cm.__exit__(None, None, None)

        # ================= P4: cross attention projections =================
        ln_cm, lnp = openp(name="psLN1", bufs=2, space="PSUM")
        A_b, B_b = ln_stats(resid, lnp)
        ln_cm.__exit__(None, None, None)

        k2_cm, k2p = openp(name="k2", bufs=NCT, side="right")
        v2_cm, v2p = openp(name="v2", bufs=2, side="right")

        w3_cm, w3 = openp(name="w3", bufs=3)

        img_cm, imgp = openp(name="img", bufs=1)
        tok_cm, tokp = openp(name="tok4", bufs=2)
        tp_cm, tpp = openp(name="psT4", bufs=2, space="PSUM")
        imgT = imgp.tile([P, NCT, TI], F8, tag="imgT", name="imgT")
        for tt in range(TI // P):
            tok = tokp.tile([P, C], F32, tag="tok", name="tok")
            nc.sync.dma_start(out=tok, in_=dr["x_img_feats"].ap()[tt * P:(tt + 1) * P, :])
            for c in range(NCT):
                tps = tpp.tile([P, P], F32, tag="tp", name="tp")
                nc.tensor.transpose(tps, tok[:, c * P:(c + 1) * P], ident)
                nc.vector.tensor_copy(out=imgT[:, c, tt * P:(tt + 1) * P], in_=tps)
        tp_cm.__exit__(None, None, None)
        tok_cm.__exit__(None, None, None)

        acc_cm, accp = openp(name="psAC4", bufs=2, space="PSUM")
        k2_t = []
        for mc in range(4):
            wt = load_wp("k_p", mc, w3)
            for mh in range(2):
                f = 2 * mc + mh
                kt = k2p.tile([P, TI], BF16, tag="k2", name="k2")
                kps = accp.tile([P, 256], F32, tag="acc256", name="acc256")
                dr_group(kps, [(wt[:, kk, :, 128 * mh:128 * mh + 128],
                                imgT[:, 2 * kk:2 * kk + 2, :]) for kk in range(4)])
                nc.scalar.activation(out=kt, in_=kps, func=AF.Identity,
                                     bias=bk_c[:, f:f + 1], scale=WSI)
                k2_t.append(kt)

        brow_v2 = rowp.tile([1, C], F32, tag="row", name="braw2")
        nc.sync.dma_start(out=brow_v2, in_=dr["bv"].ap().rearrange("(a c) -> a c", a=1))
        wv2_cm, wv2 = openp(name="wv2", bufs=3)
        bvb2 = bcast_row(brow_v2, wv2, accp, "bvb2")

        v2aug = [v2p.tile([P, 16 * 65], F32R, tag="va2", name="va2")
                 for _ in range(TI // P)]
        for cc in range(4):
            wt = load_wp("v2_p", cc, wv2)
            for st in range(TI // P):
                vps = accp.tile([P, 256], F32, tag="acc256", name="acc256")
                dr_group(vps, [(imgT[:, 2 * kk:2 * kk + 2, st * P:(st + 1) * P],
                                wt[:, kk, :, :]) for kk in range(4)])
                dst = v2aug[st].rearrange("p (h x) -> p h x", x=65)[:, 4 * cc:4 * (cc + 1), 0:64]
                nc.vector.scalar_tensor_tensor(
                    out=dst, in0=vps.rearrange("p (h x) -> p h x", x=64),
                    scalar=WSI,
                    in1=bvb2[:, 256 * cc:256 * (cc + 1)].rearrange("p (h x) -> p h x", x=64),
                    op0=OP.mult, op1=OP.add)
        for st in range(TI // P):
            nc.vector.tensor_copy(
                out=v2aug[st].rearrange("p (h x) -> p h x", x=65)[:, :, 64:65],
                in_=ones_col.rearrange("p (h x) -> p h x", x=1))
        wv2_cm.__exit__(None, None, None)
        img_cm.__exit__(None, None, None)

        q2_cm, q2p = openp(name="q2", bufs=NCT, side="right")
        hb_cm, hbp = openp(name="h2", bufs=1)
        hb = hbp.tile([P, NCT, T], F8, tag="h", name="h")
        ln_apply_f8(resid, A_b, B_b, g1, b1, hb, None, ln1z)
        q2_t = []
        for mc in range(4):
            wt = load_wp("q_p", mc, w3)
            for mh in range(2):
                f = 2 * mc + mh
                qt = q2p.tile([P, T], BF16, tag="q2", name="q2")
                for tch in range(2):
                    aps = accp.tile([P, 512], F32, tag="acc", name="acc")
                    dr_group(aps, ws_passes(wt, None, hb, None,
                                            slice(128 * mh, 128 * mh + 128),
                                            slice(512 * tch, 512 * (tch + 1))))
                    nc.scalar.activation(out=qt[:, 512 * tch:512 * (tch + 1)], in_=aps,
                                         func=AF.Identity, bias=bq_c[:, f:f + 1], scale=WSI)
                q2_t.append(qt)
        hb_cm.__exit__(None, None, None)
        acc_cm.__exit__(None, None, None)
        w3_cm.__exit__(None, None, None)

        # ================= P5: cross attention =================
        o_cm, opool = openp(name="o2", bufs=1)
        o2_all = opool.tile([P, NCT, T], F8, tag="ot", name="ot")
        pp_cm, pp = openp(name="pp2", bufs=4)
        psS_cm, psS = openp(name="psS2", bufs=2, space="PSUM")

        def kq_cross(h):
            return (k2_t[h // 2], (h % 2) * D), (q2_t[h // 2], (h % 2) * D)

        for tch in range(2):
            for h in range(H):
                attn_chunk(kq_cross, v2aug, TI // P, h, tch, psS, pp,
                           causal=False, o_all=o2_all, oD_all=None)

        psS_cm.__exit__(None, None, None)
        pp_cm.__exit__(None, None, None)
        q2_cm.__exit__(None, None, None)
        v2_cm.__exit__(None, None, None)
        k2_cm.__exit__(None, None, None)

        # ================= P6: cproj + residual (x2, in place) =================
        w4_cm, w4 = openp(name="w4", bufs=3)
        acc_cm, accp = openp(name="psAC5", bufs=3, space="PSUM")
        for mc in range(4):
            wt = load_wp("cproj_p", mc, w4)
            for mh in range(2):
                co = 2 * mc + mh
                for tch in range(2):
                    sl = slice(512 * tch, 512 * (tch + 1))
                    aps = accp.tile([P, 512], F32, tag="acc", name="acc")
                    dr_group(aps, ws_passes(wt, None, o2_all, None,
                                            slice(128 * mh, 128 * mh + 128), sl))
                    nc.vector.scalar_tensor_tensor(
                        out=resid[co][:, sl], in0=aps, scalar=WSI,
                        in1=resid[co][:, sl], op0=OP.mult, op1=OP.add)
                    if cp_bias:
                        nc.vector.tensor_scalar_add(
                            out=resid[co][:, sl], in0=resid[co][:, sl],
                            scalar1=bcp_c[:, co:co + 1])
        acc_cm.__exit__(None, None, None)
        w4_cm.__exit__(None, None, None)
        o_cm.__exit__(None, None, None)

        # ================= P7: MLP =================
        ln_cm, lnp = openp(name="psLN2", bufs=2, space="PSUM")
        A_b, B_b = ln_stats(resid, lnp)
        ln_cm.__exit__(None, None, None)

        up_cm, up = openp(name="u", bufs=16, side="right")
        h2_cm, h2p = openp(name="h3", bufs=1)
        h2 = h2p.tile([P, NCT, T], F8, tag="h", name="h")
        hd2 = h2p.tile([P, NCT, T], F8, tag="hd", name="hd")
        ln_apply_f8(resid, A_b, B_b, g2, b2, h2, hd2, ln2z)

        utiles = [up.tile([P, 2, T], F8, tag="u", name="u") for _ in range(16)]
        ud = [up.tile([P, 2, T], F8, tag="udt", name="udt") for _ in range(16)]
        uscrp_cm, uscrp = openp(name="uscr", bufs=4)
        w5_cm, w5 = openp(name="w5", bufs=4)
        accU_cm, accU = openp(name="psU", bufs=3, space="PSUM")
        for mc in range(16):
            wt = load_wp("fc_p", mc, w5)
            wtR = load_wp("fcR_p", mc, w5)
            for mh in range(2):
                ff = 2 * mc + mh
                for tch in range(2):
                    sl = slice(512 * tch, 512 * (tch + 1))
                    ups = accU.tile([P, 512], F32, tag="acc", name="acc")
                    dr_group(ups, ws_passes(wt, wtR, h2, hd2,
                                            slice(128 * mh, 128 * mh + 128), sl))
                    uscr = uscrp.tile([P, 512], F32, tag="us", name="us")
                    nc.scalar.activation(out=uscr, in_=ups,
                                         func=AF.Gelu_apprx_tanh,
                                         bias=bfc_c[:, ff:ff + 1], scale=WSI)
                    nc.vector.tensor_copy(out=utiles[ff // 2][:, ff % 2, sl], in_=uscr)
                    nc.gpsimd.scalar_tensor_tensor(
                        out=ud[ff // 2][:, ff % 2, sl],
                        in0=utiles[ff // 2][:, ff % 2, sl], scalar=-1.0,
                        in1=uscr, op0=OP.mult, op1=OP.add)
        accU_cm.__exit__(None, None, None)
        w5_cm.__exit__(None, None, None)
        uscrp_cm.__exit__(None, None, None)
        h2_cm.__exit__(None, None, None)

        w6_cm, w6 = openp(name="w6", bufs=2)
        psM_cm, psM = openp(name="psM", bufs=3, space="PSUM")
        for mc in range(4):
            wt = load_wp("mproj_p", mc, w6)
            wtR = load_wp("mprojR_p", mc, w6)
            for mh in range(2):
                co = 2 * mc + mh
                msl = slice(128 * mh, 128 * mh + 128)
                for tch in range(2):
                    sl = slice(512 * tch, 512 * (tch + 1))
                    mps = psM.tile([P, 512], F32, tag="m", name="m")
                    ps = ([(wt[:, kk, :, msl], utiles[kk][:, :, sl]) for kk in range(16)]
                          + [(wt[:, kk, :, msl], ud[kk][:, :, sl]) for kk in range(16)]
                          + [(wtR[:, kk, :, msl], utiles[kk][:, :, sl]) for kk in range(16)])
                    dr_group(mps, ps)
                    nc.vector.scalar_tensor_tensor(
                        out=resid[co][:, sl], in0=mps, scalar=WSI,
                        in1=resid[co][:, sl], op0=OP.mult, op1=OP.add)
                    if mp_bias:
                        nc.vector.tensor_scalar_add(
                            out=resid[co][:, sl], in0=resid[co][:, sl],
                            scalar1=bmp_c[:, co:co + 1])
        psM_cm.__exit__(None, None, None)
        w6_cm.__exit__(None, None, None)
        up_cm.__exit__(None, None, None)

        # ================= P8: transpose back & store =================
        tok_cm, tokp = openp(name="tok7", bufs=2)
        tp_cm, tpp = openp(name="psT7", bufs=4, space="PSUM")
        for tt in range(NTT):
            otok = tokp.tile([P, C], F32, tag="tok", name="tok")
            for c in range(NCT):
                tps = tpp.tile([P, P], F32R, tag="tpr", name="tpr")
                nc.tensor.transpose(tps, resid[c][:, tt * P:(tt + 1) * P], identR)
                if c % 2:
                    nc.vector.tensor_copy(out=otok[:, c * P:(c + 1) * P], in_=tps)
                else:
                    nc.scalar.copy(out=otok[:, c * P:(c + 1) * P], in_=tps)
            nc.sync.dma_start(out=out_d.ap()[tt * P:(tt + 1) * P, :], in_=otok)
        tp_cm.__exit__(None, None, None)
        tok_cm.__exit__(None, None, None)
        res_cm.__exit__(None, None, None)

        for cm in reversed(kw_cms):
            cm.__exit__(None, None, None)

    nc.compile()
    return nc


def _pack_core(Wq, Mc):
    K, M = Wq.shape
    KK, MC = K // 256, M // Mc
    A = Wq.reshape(KK, 2, P, MC, Mc).transpose(3, 2, 0, 1, 4)
    return np.ascontiguousarray(A.reshape(MC, P, KK * 2 * Mc))


def _pack_w(W, Mc=256):
    import ml_dtypes
    return _pack_core((np.asarray(W, np.float32) * WS).astype(ml_dtypes.float8_e4m3), Mc)


def _pack_wr(W, Mc=256):
    import ml_dtypes
    Ws = np.asarray(W, np.float32) * WS
    W8 = Ws.astype(ml_dtypes.float8_e4m3)
    R = Ws - W8.astype(np.float32)
    return _pack_core(R.astype(ml_dtypes.float8_e4m3), Mc)


def kernel(**inputs):
    from concourse.bass_utils import run_bass_kernel_spmd

    np_inputs = {k: np.asarray(v, dtype=np.float32) for k, v in inputs.items()}
    flags = (bool(np.any(np_inputs["b_aproj"])), bool(np.any(np_inputs["bcproj"])),
             bool(np.any(np_inputs["b_mproj"])),
             not np.any(np_inputs["ln1_b"]), not np.any(np_inputs["ln2_b"]))
    key = ("nc", flags)
    if key not in _CACHED:
        _CACHED[key] = _build(flags)
    nc = _CACHED[key]

    packed = {
        "attn_p": _pack_w(np_inputs["W_attn"]),
        "vR_p": _pack_wr(np_inputs["W_attn"][:, 2 * C:3 * C]),
        "aproj_p": _pack_w(np_inputs["W_aproj"]),
        "aprojR_p": _pack_wr(np_inputs["W_aproj"]),
        "q_p": _pack_w(np_inputs["Wq"]),
        "k_p": _pack_w(np_inputs["Wk"]),
        "v2_p": _pack_w(np_inputs["Wv"]),
        "cproj_p": _pack_w(np_inputs["Wcproj"]),
        "fc_p": _pack_w(np_inputs["W_fc"]),
        "fcR_p": _pack_wr(np_inputs["W_fc"]),
        "mproj_p": _pack_w(np_inputs["W_mproj"]),
        "mprojR_p": _pack_wr(np_inputs["W_mproj"]),
    }
    small = {k: np_inputs[k] for k in
             ("ln1_g", "ln1_b", "ln2_g", "ln2_b", "b_attn", "b_aproj",
              "bq", "bk", "bv", "bcproj", "b_fc", "b_mproj")}
    in_maps = []
    for b in range(B):
        m = dict(small)
        m.update(packed)
        m["x"] = np.ascontiguousarray(np_inputs["x"][b])
        m["x_img_feats"] = np.ascontiguousarray(np_inputs["x_img_feats"][b])
        in_maps.append(m)
    res = run_bass_kernel_spmd(nc, in_maps, core_ids=list(range(B)))
    out = np.stack([res.results[b]["out"] for b in range(B)], axis=0)
    return out.astype(np.float32)


# revision 8
# speedup vs baseline: 1.3106x; 1.1409x over previous
"""Trainium2 Bass kernel for a dense transformer block (self-attn + cross-attn + MLP).

Sharding: data-parallel over batch, one batch element per NeuronCore (B=8, 8 cores),
no collectives. Activations are feature-major ([C, T]) on chip.

LayerNorm is FOLDED into the projection weights on the host:
    W' = g*W - colmean(g*W),  bias' = b + W^T ln_b
so projections consume the RAW residual x (quantized straight off the stream,
no LN-apply pass); the per-token scale A[t] = 1/(128*std[t]) is applied at PSUM
evacuation (a [128,T] broadcast tile built once per LN via a ones-matmul).

Precision plan (validated in a calibrated numpy emulator, rel err ~3.8e-3):
  q,k, cross q2/k2/v2, cproj:  1-pass fp8e4m3 DoubleRow (weights x128 on host)
  v, fc:                       3-pass DoubleRow at one PSUM scale:
                               W8*x8 + W8*dx8 + R8*x8  (~bf16 accuracy, 0.75x
                               bf16 PE cost); dx8 = fp8(x - x8) via subnormals
  aproj, mproj:                bf16 (their inputs o / u are produced bf16
                               directly, avoiding on-chip hi/lo splits)
Attention interior: q/k bf16, exp/P f32r, V-aug f32r with ones column for the
softmax denominator; causal masking via one precomputed [128,896] master mask.
Softmax exp runs on paired PSUM banks ([128,1024] per ACT op); masks, squares
and fp8 deltas run on GPSIMD to keep DVE available for PSUM-coupled work.
The residual stream lives in SBUF for the whole kernel.
"""

import sys
import numpy as np

sys.path.insert(0, "/opt/trn_rl_repo")

B, T, C = 8, 1024, 1024
H = 16
D = C // H          # 64
TI = 256
FF = 4 * C          # 4096
EPS = 1e-5
NCT = C // 128      # 8 c tiles
NTT = T // 128      # 8 t tiles
P = 128
WS = 128.0          # fp8 weight scale
WSI = 1.0 / WS

_CACHED = {}

# fp8 DR packs: [MC, 128, KK*2*Mc], elem [mc, p, (kk, ko, m)] =
# q8(WS*W)[256*kk + 128*ko + p, mc*Mc + m]; *R_p carry fp8(WS*W - deq(W8)).
WPACK = {
    "attn_p": (C, 3 * C, 256),
    "vR_p": (C, C, 256),
    "q_p": (C, C, 256),
    "k_p": (C, C, 256),
    "v2_p": (C, C, 256),
    "cproj_p": (C, C, 256),
    "fc_p": (C, FF, 256),
    "fcR_p": (C, FF, 256),
}
# bf16 stationary packs: [M//128, 128, (K//128)*128], elem [mc, p, (c, m)] =
# bf16(W)[128*c + p, 128*mc + m]
BPACK = {"aproj_b": (C, C), "mproj_b": (FF, C)}


def _build(flags):
    import concourse.tile as tile
    from concourse import bacc, mybir
    from concourse.masks import make_identity

    F32, F32R = mybir.dt.float32, mybir.dt.float32r
    BF16 = mybir.dt.bfloat16
    F8 = mybir.dt.float8e4
    AF = mybir.ActivationFunctionType
    OP = mybir.AluOpType
    DR = mybir.MatmulPerfMode.DoubleRow

    qk_bias, q2_bias, ab_bias, cp_bias, mp_bias = flags

    nc = bacc.Bacc("TRN2", target_bir_lowering=False, debug=False, num_devices=8)

    dr = {}
    dr["x"] = nc.dram_tensor("x", [T, C], F32, kind="ExternalInput")
    dr["x_img_feats"] = nc.dram_tensor("x_img_feats", [TI, C], F32, kind="ExternalInput")
    for nm, shp in [
        ("b_attn", [3 * C]), ("b_aproj", [C]),
        ("bq", [C]), ("bk", [C]), ("bv", [C]), ("bcproj", [C]),
        ("b_fc", [FF]), ("b_mproj", [C]),
    ]:
        dr[nm] = nc.dram_tensor(nm, shp, F32, kind="ExternalInput")
    for nm, (K, M, Mc) in WPACK.items():
        dr[nm] = nc.dram_tensor(nm, [M // Mc, P, (K // 256) * 2 * Mc], F8,
                                kind="ExternalInput")
    for nm, (K, M) in BPACK.items():
        dr[nm] = nc.dram_tensor(nm, [M // P, P, (K // P) * P], BF16,
                                kind="ExternalInput")
    out_d = nc.dram_tensor("out", [T, C], F32, kind="ExternalOutput")

    with tile.TileContext(nc) as tc, nc.allow_low_precision(
        reason="fp8 DoubleRow projections + bf16 attention are intentional"
    ):
        kw_cms = []

        def openp(**kw):
            cm = tc.tile_pool(**kw)
            return cm, cm.__enter__()

        def openkw(**kw):
            cm, p = openp(**kw)
            kw_cms.append(cm)
            return p

        # ---------------- kernel-wide pools (left-stack base) ----------------
        constp = openkw(name="const", bufs=1)
        fsrp = openkw(name="fsr", bufs=2)       # f32r [128,512] squares
        abp = openkw(name="ab", bufs=1)         # A_b [128,1024] + A_col
        rowp = openkw(name="rows", bufs=4)      # [1,1024] rows
        rbp = openkw(name="rb", bufs=3)         # [64,512] + [1,512] rden

        # ---------------- constants ----------------
        ident = constp.tile([P, P], F32)
        make_identity(nc, ident)
        identR = constp.tile([P, P], F32R)
        nc.vector.tensor_copy(out=identR, in_=ident)

        ones_col = constp.tile([P, 16], F32)
        nc.vector.memset(ones_col, 1.0)
        ones128R = constp.tile([P, 1], F32R)
        nc.vector.tensor_copy(out=ones128R, in_=ones_col[:, 0:1])
        o1x = constp.tile([1, P], F32)
        nc.vector.memset(o1x, 1.0)
        ones_1x128 = constp.tile([1, P], F32R)
        nc.vector.tensor_copy(out=ones_1x128, in_=o1x)
        epsS_t = constp.tile([1, 1], F32)
        nc.vector.memset(epsS_t, EPS * WS * WS)
        zeros384 = constp.tile([P, 384], F32)
        nc.vector.memset(zeros384, 0.0)

        master = constp.tile([P, 896], F32)
        nc.gpsimd.memset(master, 1.0)
        nc.gpsimd.affine_select(
            out=master, in_=master, compare_op=OP.is_ge, fill=0.0,
            base=-384, pattern=[[1, 896]], channel_multiplier=-1)

        # ================= P0: load & transpose x (issued first) =============
        res_cm, residp = openp(name="resid", bufs=NCT, side="right")
        resid = [residp.tile([P, T], F32R, tag="res", name="res") for _ in range(NCT)]

        tok_cm, tokp = openp(name="tok0", bufs=2)
        tp_cm, tpp = openp(name="psT0", bufs=2, space="PSUM")
        for tt in range(NTT):
            tok = tokp.tile([P, C], F32, tag="tok", name="tok")
            nc.sync.dma_start(out=tok, in_=dr["x"].ap()[tt * P:(tt + 1) * P, :])
            for c in range(NCT):
                tps = tpp.tile([P, P], F32, tag="tp", name="tp")
                nc.tensor.transpose(tps, tok[:, c * P:(c + 1) * P], ident)
                if c % 2:
                    nc.vector.tensor_copy(out=resid[c][:, tt * P:(tt + 1) * P], in_=tps)
                else:
                    nc.scalar.copy(out=resid[c][:, tt * P:(tt + 1) * P], in_=tps)
        tp_cm.__exit__(None, None, None)
        tok_cm.__exit__(None, None, None)

        # ---------------- small input rows (issued after x) ----------------
        def load_cols(name, nf):
            t = constp.tile([P, nf], F32, name=name + "_c")
            nc.sync.dma_start(out=t, in_=dr[name].ap().rearrange("(f p) -> p f", p=P))
            return t

        bqk = constp.tile([P, 16], F32)
        nc.sync.dma_start(out=bqk, in_=dr["b_attn"].ap()[0:2 * C].rearrange("(f p) -> p f", p=P))
        bq_c = load_cols("bq", NCT)
        bk_c = load_cols("bk", NCT)
        bap_c = load_cols("b_aproj", NCT)
        bcp_c = load_cols("bcproj", NCT)
        bmp_c = load_cols("b_mproj", NCT)
        bfc_c = load_cols("b_fc", FF // 128)

        # ---------------- helpers ----------------
        def load_wp(name, mc, wpool):
            K, M, Mc = WPACK[name]
            KK = K // 256
            t = wpool.tile([P, KK, 2, Mc], F8, tag="wp", name="wp")
            nc.sync.dma_start(
                out=t,
                in_=dr[name].ap()[mc].rearrange("p (kk ko m) -> p kk ko m", kk=KK, ko=2))
            return t

        def load_wb(name, mc, wpool):
            K, M = BPACK[name]
            t = wpool.tile([P, K // P, P], BF16, tag="wb", name="wb")
            nc.sync.dma_start(
                out=t, in_=dr[name].ap()[mc].rearrange("p (c m) -> p c m", m=P))
            return t

        def bcast_row(row_f32, dest_pool, psp, tag):
            rowr = rowp.tile([1, C], F32R, tag="row", name="rowr")
            nc.vector.tensor_copy(out=rowr, in_=row_f32)
            dest = dest_pool.tile([P, C], F32, tag=tag, name=tag)
            for cc in range(2):
                bps = psp.tile([P, 512], F32, tag="bc", name="bc")
                nc.tensor.matmul(bps, ones_1x128, rowr[:, 512 * cc:512 * (cc + 1)],
                                 start=True, stop=True)
                nc.scalar.copy(out=dest[:, 512 * cc:512 * (cc + 1)], in_=bps)
            return dest

        def ln_stats(xtiles, psp, with_col=False):
            """A_b [128,T] broadcast of A[t] = 1/(128*std[t]); opt A_col [128,NTT]."""
            sum_ps, sq_ps = [], []
            for tch in range(2):
                sp = psp.tile([1, 512], F32, tag="lnsum", name="lnsum")
                qp = psp.tile([1, 512], F32, tag="lnsq", name="lnsq")
                for c in range(NCT):
                    xs = xtiles[c][:, 512 * tch:512 * (tch + 1)]
                    nc.tensor.matmul(sp, ones128R, xs, start=(c == 0), stop=(c == NCT - 1))
                    sq = fsrp.tile([P, 512], F32R, tag="sq", name="sq")
                    nc.gpsimd.tensor_tensor(out=sq, in0=xs, in1=xs, op=OP.mult)
                    nc.tensor.matmul(qp, ones128R, sq, start=(c == 0), stop=(c == NCT - 1))
                sum_ps.append(sp)
                sq_ps.append(qp)
            mu = rowp.tile([1, T], F32, tag="row", name="mu")
            for tch in range(2):
                nc.vector.tensor_scalar_mul(out=mu[:, 512 * tch:512 * (tch + 1)],
                                            in0=sum_ps[tch], scalar1=1.0 / C)
            musq = rowp.tile([1, T], F32, tag="row", name="musq")
            nc.vector.tensor_tensor(out=musq, in0=mu, in1=mu, op=OP.mult)
            msq = rowp.tile([1, T], F32, tag="row", name="msq")
            for tch in range(2):
                sl = slice(512 * tch, 512 * (tch + 1))
                nc.vector.scalar_tensor_tensor(
                    out=msq[:, sl], in0=sq_ps[tch], scalar=1.0 / C,
                    in1=musq[:, sl], op0=OP.mult, op1=OP.subtract)
            nc.scalar.activation(out=musq, in_=msq, func=AF.Sqrt, bias=epsS_t,
                                 scale=WS * WS)
            arow = rowp.tile([1, T], F32R, tag="row", name="arow")
            nc.vector.reciprocal(out=arow, in_=musq)
            A_b = abp.tile([P, T], F32, tag="A_b", name="A_b")
            for tch in range(2):
                sl = slice(512 * tch, 512 * (tch + 1))
                bps = psp.tile([P, 512], F32, tag="bc", name="bc")
                nc.tensor.matmul(bps, ones_1x128, arow[:, sl], start=True, stop=True)
                nc.scalar.copy(out=A_b[:, sl], in_=bps)
            if not with_col:
                return A_b, None
            A_col = abp.tile([P, NTT], F32, tag="A_col", name="A_col")
            for tt in range(NTT):
                cps = psp.tile([P, P], F32, tag="bc", name="bc")
                nc.tensor.transpose(cps, A_b[:, tt * P:(tt + 1) * P], ident)
                nc.vector.tensor_copy(out=A_col[:, tt:tt + 1], in_=cps[:, 0:1])
            return A_b, A_col

        def quant_x(xtiles, x8, xd8):
            """fp8 copy of the residual stream (+ optional fp8 delta)."""
            for c in range(NCT):
                nc.scalar.copy(out=x8[:, c, :], in_=xtiles[c])
                if xd8 is not None:
                    nc.gpsimd.scalar_tensor_tensor(
                        out=xd8[:, c, :], in0=x8[:, c, :], scalar=-1.0,
                        in1=xtiles[c], op0=OP.mult, op1=OP.add)

        def attn_chunk(kq_of, vaug_tiles, n_s, h, tch, psp, ppool, causal,
                       o_all, o_dt):
            (kt, ko), (qt, qo) = kq_of(h)
            tsl = slice(512 * tch, 512 * (tch + 1))
            ptiles = []
            pair_ps = []
            for pr in range(n_s // 2):
                sps = psp.tile([P, 1024], F32, tag="s", name="s")
                for hf in range(2):
                    st = 2 * pr + hf
                    nc.tensor.matmul(sps[:, 512 * hf:512 * hf + 512],
                                     kt[ko:ko + D, st * P:(st + 1) * P],
                                     qt[qo:qo + D, tsl], start=True, stop=True,
                                     tile_position=(ko, 0))
                pair_ps.append(sps)
            for pr in range(n_s // 2):
                sps = pair_ps[pr]
                pt = ppool.tile([P, 1024], F32R, tag="p", name="p")
                j0 = 2 * pr - 4 * tch
                j1 = j0 + 1
                d0 = causal and j0 >= 0
                d1 = causal and j1 >= 0
                z0 = P * j0 if d0 else 0
                z1 = P * j1 if d1 else 0
                nc.scalar.activation(out=pt[:, z0:1024], in_=sps[:, z0:1024],
                                     func=AF.Exp, scale=0.125)
                if d0 and z0:
                    nc.gpsimd.tensor_copy(out=pt[:, 0:z0], in_=zeros384[:, 0:z0])
                if d1 and z1:
                    nc.gpsimd.tensor_copy(out=pt[:, 512:512 + z1], in_=zeros384[:, 0:z1])
                if d0:
                    nc.gpsimd.tensor_tensor(out=pt[:, z0:z0 + P], in0=pt[:, z0:z0 + P],
                                            in1=master[:, 384:512], op=OP.mult)
                if d1:
                    nc.gpsimd.tensor_tensor(out=pt[:, 512 + z1:512 + z1 + P],
                                            in0=pt[:, 512 + z1:512 + z1 + P],
                                            in1=master[:, 384:512], op=OP.mult)
                ptiles.append(pt)
            ops = psp.tile([65, 512], F32, tag="o", name="o")
            for st in range(n_s):
                pt = ptiles[st // 2][:, 512 * (st % 2):512 * (st % 2) + 512]
                nc.tensor.matmul(ops, vaug_tiles[st][:, 65 * h:65 * h + 65],
                                 pt, start=(st == 0), stop=(st == n_s - 1))
            rden = rbp.tile([1, 512], F32R, tag="rden", name="rden")
            nc.vector.reciprocal(out=rden, in_=ops[64:65, :])
            bps = psp.tile([64, 512], F32, tag="b", name="b")
            nc.tensor.matmul(bps, ones_1x128[:, 0:64], rden, start=True, stop=True)
            rb = rbp.tile([64, 512], F32, tag="rb", name="rb")
            nc.scalar.copy(out=rb, in_=bps)
            po = (h % 2) * D
            nc.vector.tensor_tensor(out=o_all[po:po + D, h // 2, tsl],
                                    in0=ops[0:64, :], in1=rb, op=OP.mult)

        def dr_group(psum, pairs):
            n = len(pairs)
            for i, (lh, rh) in enumerate(pairs):
                nc.tensor.matmul(psum, lh, rh, start=(i == 0), stop=(i == n - 1),
                                 perf_mode=DR)

        def ws_passes(wt, wtR, h8, hd, msl, tsl2):
            ps = [(wt[:, kk, :, msl], h8[:, 2 * kk:2 * kk + 2, tsl2]) for kk in range(4)]
            if hd is not None:
                ps += [(wt[:, kk, :, msl], hd[:, 2 * kk:2 * kk + 2, tsl2]) for kk in range(4)]
            if wtR is not None:
                ps += [(wtR[:, kk, :, msl], h8[:, 2 * kk:2 * kk + 2, tsl2]) for kk in range(4)]
            return ps

        # ================= P1: LN1 + qkv projections =================
        x8_cm, x8p = openp(name="x8", bufs=1)
        x8 = x8p.tile([P, NCT, T], F8, tag="x8", name="x8")
        xd8 = x8p.tile([P, NCT, T], F8, tag="xd8", name="xd8")
        quant_x(resid, x8, xd8)

        ln_cm, lnp = openp(name="psLN0", bufs=2, space="PSUM")
        A_b, A_col = ln_stats(resid, lnp, with_col=True)
        ln_cm.__exit__(None, None, None)

        vap_cm, vap = openp(name="vaug", bufs=NTT, side="right")
        vaug = [vap.tile([P, 16 * 65], F32R, tag="va", name="va") for _ in range(NTT)]

        wv_cm, wv = openp(name="wv", bufs=4)
        accv_cm, accv = openp(name="psACv", bufs=2, space="PSUM")
        brow_v = rowp.tile([1, C], F32, tag="row", name="braw")
        nc.sync.dma_start(out=brow_v,
                          in_=dr["b_attn"].ap()[2 * C:3 * C].rearrange("(a c) -> a c", a=1))
        bvb1 = bcast_row(brow_v, wv, accv, "bvb")
        for cc in range(4):   # v output chunks of 256 cols (4 heads each)
            wt = load_wp("attn_p", 8 + cc, wv)
            wtR = load_wp("vR_p", cc, wv)
            for tt in range(NTT):
                vps = accv.tile([P, 256], F32, tag="acc", name="acc")
                tsl = slice(tt * P, (tt + 1) * P)
                ps = ([(x8[:, 2 * kk:2 * kk + 2, tsl], wt[:, kk, :, :]) for kk in range(4)]
                      + [(xd8[:, 2 * kk:2 * kk + 2, tsl], wt[:, kk, :, :]) for kk in range(4)]
                      + [(x8[:, 2 * kk:2 * kk + 2, tsl], wtR[:, kk, :, :]) for kk in range(4)])
                dr_group(vps, ps)
                dst = vaug[tt].rearrange("p (h x) -> p h x", x=65)[:, 4 * cc:4 * (cc + 1), 0:64]
                nc.vector.scalar_tensor_tensor(
                    out=dst, in0=vps.rearrange("p (h x) -> p h x", x=64),
                    scalar=A_col[:, tt:tt + 1],
                    in1=bvb1[:, 256 * cc:256 * (cc + 1)].rearrange("p (h x) -> p h x", x=64),
                    op0=OP.mult, op1=OP.add)
        for tt in range(NTT):
            nc.vector.tensor_copy(
                out=vaug[tt].rearrange("p (h x) -> p h x", x=65)[:, :, 64:65],
                in_=ones_col.rearrange("p (h x) -> p h x", x=1))

        qk_cm, qkp = openp(name="qk", bufs=16, side="right")
        w1_cm, w1 = openp(name="w1", bufs=3)
        acc_cm, accp = openp(name="psAC1", bufs=3, space="PSUM")
        qk_t = []
        for mc in range(8):
            wt = load_wp("attn_p", mc, w1)
            for mh in range(2):
                f = 2 * mc + mh
                qt = qkp.tile([P, T], BF16, tag="qk", name="qk")
                for tch in range(2):
                    sl = slice(512 * tch, 512 * (tch + 1))
                    aps = accp.tile([P, 512], F32, tag="acc", name="acc")
                    dr_group(aps, ws_passes(wt, None, x8, None,
                                            slice(128 * mh, 128 * mh + 128), sl))
                    nc.vector.tensor_tensor(out=qt[:, sl], in0=aps, in1=A_b[:, sl],
                                            op=OP.mult)
                    if qk_bias:
                        nc.vector.tensor_scalar_add(out=qt[:, sl], in0=qt[:, sl],
                                                    scalar1=bqk[:, f:f + 1])
                qk_t.append(qt)
        acc_cm.__exit__(None, None, None)
        w1_cm.__exit__(None, None, None)
        accv_cm.__exit__(None, None, None)
        wv_cm.__exit__(None, None, None)
        x8_cm.__exit__(None, None, None)

        # ================= P2: self attention =================
        o_cm, opool = openp(name="o1", bufs=1)
        o_all = opool.tile([P, NCT, T], BF16, tag="ot", name="ot")
        pp_cm, pp = openp(name="pp1", bufs=5)
        psS_cm, psS = openp(name="psS1", bufs=2, space="PSUM")

        def kq_self(h):
            return (qk_t[8 + h // 2], (h % 2) * D), (qk_t[h // 2], (h % 2) * D)

        for tch in range(2):
            for h in range(H):
                attn_chunk(kq_self, vaug, 4 * (tch + 1), h, tch, psS, pp,
                           causal=True, o_all=o_all, o_dt=BF16)

        psS_cm.__exit__(None, None, None)
        pp_cm.__exit__(None, None, None)
        qk_cm.__exit__(None, None, None)
        vap_cm.__exit__(None, None, None)

        # ================= P3: aproj (bf16) + residual in place ======
        w2_cm, w2 = openp(name="w2", bufs=3)
        acc_cm, accp = openp(name="psAC3", bufs=3, space="PSUM")
        for co in range(NCT):
            wt = load_wb("aproj_b", co, w2)
            for tch in range(2):
                sl = slice(512 * tch, 512 * (tch + 1))
                aps = accp.tile([P, 512], F32, tag="acc", name="acc")
                for c in range(NCT):
                    nc.tensor.matmul(aps, wt[:, c, :], o_all[:, c, sl],
                                     start=(c == 0), stop=(c == NCT - 1))
                nc.vector.tensor_tensor(out=resid[co][:, sl], in0=aps,
                                        in1=resid[co][:, sl], op=OP.add)
                if ab_bias:
                    nc.vector.tensor_scalar_add(
                        out=resid[co][:, sl], in0=resid[co][:, sl],
                        scalar1=bap_c[:, co:co + 1])
        acc_cm.__exit__(None, None, None)
        w2_cm.__exit__(None, None, None)
        o_cm.__exit__(None, None, None)

        # ================= P4: cross attention projections =================
        x1_cm, x1p = openp(name="x18", bufs=1)
        x18 = x1p.tile([P, NCT, T], F8, tag="x8", name="x8")
        quant_x(resid, x18, None)

        ln_cm, lnp = openp(name="psLN1", bufs=2, space="PSUM")
        A_b, _ = ln_stats(resid, lnp)
        ln_cm.__exit__(None, None, None)

        k2_cm, k2p = openp(name="k2", bufs=NCT, side="right")
        v2_cm, v2p = openp(name="v2", bufs=2, side="right")

        w3_cm, w3 = openp(name="w3", bufs=3)

        img_cm, imgp = openp(name="img", bufs=1)
        tok_cm, tokp = openp(name="tok4", bufs=2)
        tp_cm, tpp = openp(name="psT4", bufs=2, space="PSUM")
        imgT = imgp.tile([P, NCT, TI], F8, tag="imgT", name="imgT")
        for tt in range(TI // P):
            tok = tokp.tile([P, C], F32, tag="tok", name="tok")
            nc.sync.dma_start(out=tok, in_=dr["x_img_feats"].ap()[tt * P:(tt + 1) * P, :])
            for c in range(NCT):
                tps = tpp.tile([P, P], F32, tag="tp", name="tp")
                nc.tensor.transpose(tps, tok[:, c * P:(c + 1) * P], ident)
                nc.vector.tensor_copy(out=imgT[:, c, tt * P:(tt + 1) * P], in_=tps)
        tp_cm.__exit__(None, None, None)
        tok_cm.__exit__(None, None, None)

        acc_cm, accp = openp(name="psAC4", bufs=2, space="PSUM")
        k2_t = []
        for mc in range(4):
            wt = load_wp("k_p", mc, w3)
            for mh in range(2):
                f = 2 * mc + mh
                kt = k2p.tile([P, TI], BF16, tag="k2", name="k2")
                kps = accp.tile([P, 256], F32, tag="acc256", name="acc256")
                dr_group(kps, [(wt[:, kk, :, 128 * mh:128 * mh + 128],
                                imgT[:, 2 * kk:2 * kk + 2, :]) for kk in range(4)])
                nc.scalar.activation(out=kt, in_=kps, func=AF.Identity,
                                     bias=bk_c[:, f:f + 1], scale=WSI)
                k2_t.append(kt)

        brow_v2 = rowp.tile([1, C], F32, tag="row", name="braw2")
        nc.sync.dma_start(out=brow_v2, in_=dr["bv"].ap().rearrange("(a c) -> a c", a=1))
        wv2_cm, wv2 = openp(name="wv2", bufs=3)
        bvb2 = bcast_row(brow_v2, wv2, accp, "bvb2")

        v2aug = [v2p.tile([P, 16 * 65], F32R, tag="va2", name="va2")
                 for _ in range(TI // P)]
        for cc in range(4):
            wt = load_wp("v2_p", cc, wv2)
            for st in range(TI // P):
                vps = accp.tile([P, 256], F32, tag="acc256", name="acc256")
                dr_group(vps, [(imgT[:, 2 * kk:2 * kk + 2, st * P:(st + 1) * P],
                                wt[:, kk, :, :]) for kk in range(4)])
                dst = v2aug[st].rearrange("p (h x) -> p h x", x=65)[:, 4 * cc:4 * (cc + 1), 0:64]
                nc.vector.scalar_tensor_tensor(
                    out=dst, in0=vps.rearrange("p (h x) -> p h x", x=64),
                    scalar=WSI,
                    in1=bvb2[:, 256 * cc:256 * (cc + 1)].rearrange("p (h x) -> p h x", x=64),
                    op0=OP.mult, op1=OP.add)
        for st in range(TI // P):
            nc.vector.tensor_copy(
                out=v2aug[st].rearrange("p (h x) -> p h x", x=65)[:, :, 64:65],
                in_=ones_col.rearrange("p (h x) -> p h x", x=1))
        wv2_cm.__exit__(None, None, None)
        img_cm.__exit__(None, None, None)

        q2_cm, q2p = openp(name="q2", bufs=NCT, side="right")
        q2_t = []
        for mc in range(4):
            wt = load_wp("q_p", mc, w3)
            for mh in range(2):
                f = 2 * mc + mh
                qt = q2p.tile([P, T], BF16, tag="q2", name="q2")
                for tch in range(2):
                    sl = slice(512 * tch, 512 * (tch + 1))
                    aps = accp.tile([P, 512], F32, tag="acc", name="acc")
                    dr_group(aps, ws_passes(wt, None, x18, None,
                                            slice(128 * mh, 128 * mh + 128), sl))
                    nc.vector.tensor_tensor(out=qt[:, sl], in0=aps, in1=A_b[:, sl],
                                            op=OP.mult)
                    if q2_bias:
                        nc.vector.tensor_scalar_add(out=qt[:, sl], in0=qt[:, sl],
                                                    scalar1=bq_c[:, f:f + 1])
                q2_t.append(qt)
        acc_cm.__exit__(None, None, None)
        w3_cm.__exit__(None, None, None)
        x1_cm.__exit__(None, None, None)

        # ================= P5: cross attention =================
        o_cm, opool = openp(name="o2", bufs=1)
        o2_all = opool.tile([P, NCT, T], F8, tag="ot", name="ot")
        pp_cm, pp = openp(name="pp2", bufs=4)
        psS_cm, psS = openp(name="psS2", bufs=2, space="PSUM")

        def kq_cross(h):
            return (k2_t[h // 2], (h % 2) * D), (q2_t[h // 2], (h % 2) * D)

        for tch in range(2):
            for h in range(H):
                attn_chunk(kq_cross, v2aug, TI // P, h, tch, psS, pp,
                           causal=False, o_all=o2_all, o_dt=F8)

        psS_cm.__exit__(None, None, None)
        pp_cm.__exit__(None, None, None)
        q2_cm.__exit__(None, None, None)
        v2_cm.__exit__(None, None, None)
        k2_cm.__exit__(None, None, None)

        # ================= P6: cproj + residual (x2, in place) =================
        w4_cm, w4 = openp(name="w4", bufs=3)
        acc_cm, accp = openp(name="psAC5", bufs=3, space="PSUM")
        for mc in range(4):
            wt = load_wp("cproj_p", mc, w4)
            for mh in range(2):
                co = 2 * mc + mh
                for tch in range(2):
                    sl = slice(512 * tch, 512 * (tch + 1))
                    aps = accp.tile([P, 512], F32, tag="acc", name="acc")
                    dr_group(aps, ws_passes(wt, None, o2_all, None,
                                            slice(128 * mh, 128 * mh + 128), sl))
                    nc.vector.scalar_tensor_tensor(
                        out=resid[co][:, sl], in0=aps, scalar=WSI,
                        in1=resid[co][:, sl], op0=OP.mult, op1=OP.add)
                    if cp_bias:
                        nc.vector.tensor_scalar_add(
                            out=resid[co][:, sl], in0=resid[co][:, sl],
                            scalar1=bcp_c[:, co:co + 1])
        acc_cm.__exit__(None, None, None)
        w4_cm.__exit__(None, None, None)
        o_cm.__exit__(None, None, None)

        # ================= P7: MLP =================
        x2_cm, x2p = openp(name="x28", bufs=1)
        x28 = x2p.tile([P, NCT, T], F8, tag="x8", name="x8")
        x2d8 = x2p.tile([P, NCT, T], F8, tag="xd8", name="xd8")
        quant_x(resid, x28, x2d8)

        ln_cm, lnp = openp(name="psLN2", bufs=2, space="PSUM")
        A_b, _ = ln_stats(resid, lnp)
        ln_cm.__exit__(None, None, None)

        up_cm, up = openp(name="u", bufs=16, side="right")
        utiles = [up.tile([P, 2, T], BF16, tag="u", name="u") for _ in range(16)]
        uscr_cm, uscrp = openp(name="uscr", bufs=4)
        w5_cm, w5 = openp(name="w5", bufs=4)
        accU_cm, accU = openp(name="psU", bufs=3, space="PSUM")
        for mc in range(16):
            wt = load_wp("fc_p", mc, w5)
            wtR = load_wp("fcR_p", mc, w5)
            for mh in range(2):
                ff = 2 * mc + mh
                for tch in range(2):
                    sl = slice(512 * tch, 512 * (tch + 1))
                    ups = accU.tile([P, 512], F32, tag="acc", name="acc")
                    dr_group(ups, ws_passes(wt, wtR, x28, x2d8,
                                            slice(128 * mh, 128 * mh + 128), sl))
                    uscr = uscrp.tile([P, 512], F32, tag="us", name="us")
                    nc.vector.tensor_tensor(out=uscr, in0=ups, in1=A_b[:, sl],
                                            op=OP.mult)
                    nc.scalar.activation(out=utiles[ff // 2][:, ff % 2, sl], in_=uscr,
                                         func=AF.Gelu_apprx_tanh,
                                         bias=bfc_c[:, ff:ff + 1], scale=1.0)
        accU_cm.__exit__(None, None, None)
        w5_cm.__exit__(None, None, None)
        uscr_cm.__exit__(None, None, None)
        x2_cm.__exit__(None, None, None)

        w6_cm, w6 = openp(name="w6", bufs=2)
        psM_cm, psM = openp(name="psM", bufs=3, space="PSUM")
        for co in range(NCT):
            wt = load_wb("mproj_b", co, w6)
            for tch in range(2):
                sl = slice(512 * tch, 512 * (tch + 1))
                mps = psM.tile([P, 512], F32, tag="m", name="m")
                for ff in range(FF // P):
                    nc.tensor.matmul(mps, wt[:, ff, :], utiles[ff // 2][:, ff % 2, sl],
                                     start=(ff == 0), stop=(ff == FF // P - 1))
                nc.vector.tensor_tensor(out=resid[co][:, sl], in0=mps,
                                        in1=resid[co][:, sl], op=OP.add)
                if mp_bias:
                    nc.vector.tensor_scalar_add(
                        out=resid[co][:, sl], in0=resid[co][:, sl],
                        scalar1=bmp_c[:, co:co + 1])
        psM_cm.__exit__(None, None, None)
        w6_cm.__exit__(None, None, None)
        up_cm.__exit__(None, None, None)

        # ================= P8: transpose back & store =================
        tok_cm, tokp = openp(name="tok7", bufs=2)
        tp_cm, tpp = openp(name="psT7", bufs=4, space="PSUM")
        for tt in range(NTT):
            otok = tokp.tile([P, C], F32, tag="tok", name="tok")
            for c in range(NCT):
                tps = tpp.tile([P, P], F32R, tag="tpr", name="tpr")
                nc.tensor.transpose(tps, resid[c][:, tt * P:(tt + 1) * P], identR)
                if c % 2:
                    nc.vector.tensor_copy(out=otok[:, c * P:(c + 1) * P], in_=tps)
                else:
                    nc.scalar.copy(out=otok[:, c * P:(c + 1) * P], in_=tps)
            nc.sync.dma_start(out=out_d.ap()[tt * P:(tt + 1) * P, :], in_=otok)
        tp_cm.__exit__(None, None, None)
        tok_cm.__exit__(None, None, None)
        res_cm.__exit__(None, None, None)

        for cm in reversed(kw_cms):
            cm.__exit__(None, None, None)

    nc.compile()
    return nc


def _pack_core(Wq, Mc):
    K, M = Wq.shape
    KK, MC = K // 256, M // Mc
    A = Wq.reshape(KK, 2, P, MC, Mc).transpose(3, 2, 0, 1, 4)
    return np.ascontiguousarray(A.reshape(MC, P, KK * 2 * Mc))


def _pack_w(W, Mc=256):
    import ml_dtypes
    return _pack_core((np.asarray(W, np.float32) * WS).astype(ml_dtypes.float8_e4m3), Mc)


def _pack_wr(W, Mc=256):
    import ml_dtypes
    Ws = np.asarray(W, np.float32) * WS
    W8 = Ws.astype(ml_dtypes.float8_e4m3)
    R = Ws - W8.astype(np.float32)
    return _pack_core(R.astype(ml_dtypes.float8_e4m3), Mc)


def _pack_b(W):
    """[K, M] f32 -> [M//128, 128, (K//128)*128] bf16 stationary slabs."""
    import ml_dtypes
    K, M = W.shape
    A = np.asarray(W, np.float32).reshape(K // P, P, M // P, P).transpose(2, 1, 0, 3)
    return np.ascontiguousarray(A.reshape(M // P, P, K).astype(ml_dtypes.bfloat16))


def _fold(W, g):
    Wg = np.asarray(W, np.float32) * np.asarray(g, np.float32)[:, None]
    return Wg - Wg.mean(0, keepdims=True)


def kernel(**inputs):
    from concourse.bass_utils import run_bass_kernel_spmd

    np_inputs = {k: np.asarray(v, dtype=np.float32) for k, v in inputs.items()}
    g1, b1 = np_inputs["ln1_g"], np_inputs["ln1_b"]
    g2, b2 = np_inputs["ln2_g"], np_inputs["ln2_b"]
    Wa = np_inputs["W_attn"]

    # fold LN gains/means into weights; fold LN biases into effective biases
    Wa_f = _fold(Wa, g1)
    Wq_f = _fold(np_inputs["Wq"], g1)
    Wfc_f = _fold(np_inputs["W_fc"], g2)
    b_attn_eff = np_inputs["b_attn"] + Wa.T @ b1
    bq_eff = np_inputs["bq"] + np_inputs["Wq"].T @ b1
    bfc_eff = np_inputs["b_fc"] + np_inputs["W_fc"].T @ b2

    flags = (bool(np.any(b_attn_eff[0:2 * C])), bool(np.any(bq_eff)),
             bool(np.any(np_inputs["b_aproj"])), bool(np.any(np_inputs["bcproj"])),
             bool(np.any(np_inputs["b_mproj"])))
    key = ("nc", flags)
    if key not in _CACHED:
        _CACHED[key] = _build(flags)
    nc = _CACHED[key]

    packed = {
        "attn_p": _pack_w(Wa_f),
        "vR_p": _pack_wr(Wa_f[:, 2 * C:3 * C]),
        "q_p": _pack_w(Wq_f),
        "k_p": _pack_w(np_inputs["Wk"]),
        "v2_p": _pack_w(np_inputs["Wv"]),
        "cproj_p": _pack_w(np_inputs["Wcproj"]),
        "fc_p": _pack_w(Wfc_f),
        "fcR_p": _pack_wr(Wfc_f),
        "aproj_b": _pack_b(np_inputs["W_aproj"]),
        "mproj_b": _pack_b(np_inputs["W_mproj"]),
    }
    small = {
        "b_attn": b_attn_eff, "bq": bq_eff, "b_fc": bfc_eff,
        "b_aproj": np_inputs["b_aproj"], "bk": np_inputs["bk"],
        "bv": np_inputs["bv"], "bcproj": np_inputs["bcproj"],
        "b_mproj": np_inputs["b_mproj"],
    }
    in_maps = []
    for b in range(B):
        m = dict(small)
        m.update(packed)
        m["x"] = np.ascontiguousarray(np_inputs["x"][b])
        m["x_img_feats"] = np.ascontiguousarray(np_inputs["x_img_feats"][b])
        in_maps.append(m)
    res = run_bass_kernel_spmd(nc, in_maps, core_ids=list(range(B)))
    out = np.stack([res.results[b]["out"] for b in range(B)], axis=0)
    return out.astype(np.float32)


# revision 17
# speedup vs baseline: 1.3496x; 1.0298x over previous
"""Trainium2 Bass kernel for a dense transformer block (self-attn + cross-attn + MLP).

Sharding: data-parallel over batch, one batch element per NeuronCore (B=8, 8 cores),
no collectives. Activations are feature-major ([C, T]) on chip.

LayerNorm is FOLDED into the projection weights on the host:
    W' = g*W - colmean(g*W),  bias' = b + W^T ln_b
so projections consume the RAW residual x (quantized straight off the stream,
no LN-apply pass); the per-token scale A[t] = 1/(128*std[t]) is applied at PSUM
evacuation (a [128,T] broadcast tile built once per LN via a ones-matmul).

Precision plan (validated in a calibrated numpy emulator, rel err ~3.8e-3):
  q,k, cross q2/k2/v2, cproj:  1-pass fp8e4m3 DoubleRow (weights x128 on host)
  v, fc:                       3-pass DoubleRow at one PSUM scale:
                               W8*x8 + W8*dx8 + R8*x8  (~bf16 accuracy, 0.75x
                               bf16 PE cost); dx8 = fp8(x - x8) via subnormals
  aproj, mproj:                bf16 (their inputs o / u are produced bf16
                               directly, avoiding on-chip hi/lo splits)
Attention interior: q/k bf16, exp/P f32r, V-aug f32r with ones column for the
softmax denominator; causal masking via one precomputed [128,896] master mask.
Softmax exp runs on paired PSUM banks ([128,1024] per ACT op); masks, squares
and fp8 deltas run on GPSIMD to keep DVE available for PSUM-coupled work.
The residual stream lives in SBUF for the whole kernel.
"""

import sys
import numpy as np

sys.path.insert(0, "/opt/trn_rl_repo")

B, T, C = 8, 1024, 1024
H = 16
D = C // H          # 64
TI = 256
FF = 4 * C          # 4096
EPS = 1e-5
NCT = C // 128      # 8 c tiles
NTT = T // 128      # 8 t tiles
P = 128
WS = 128.0          # fp8 weight scale
WSI = 1.0 / WS

_CACHED = {}

# fp8 DR packs: [MC, 128, KK*2*Mc], elem [mc, p, (kk, ko, m)] =
# q8(WS*W)[256*kk + 128*ko + p, mc*Mc + m]; *R_p carry fp8(WS*W - deq(W8)).
WPACK = {
    "attn_p": (C, 3 * C, 256),
    "vR_p": (C, C, 256),
    "q_p": (C, C, 256),
    "k_p": (C, C, 256),
    "v2_p": (C, C, 256),
    "cproj_p": (C, C, 256),
    "fc_p": (C, FF, 256),
    "fcR_p": (C, FF, 256),
}
# bf16 stationary packs: [M//128, 128, (K//128)*128], elem [mc, p, (c, m)] =
# bf16(W)[128*c + p, 128*mc + m]
BPACK = {"aproj_b": (C, C), "mproj_b": (FF, C)}


def _build(flags):
    import concourse.tile as tile
    from concourse import bacc, mybir
    from concourse.masks import make_identity

    F32, F32R = mybir.dt.float32, mybir.dt.float32r
    BF16 = mybir.dt.bfloat16
    F8 = mybir.dt.float8e4
    AF = mybir.ActivationFunctionType
    OP = mybir.AluOpType
    DR = mybir.MatmulPerfMode.DoubleRow

    qk_bias, q2_bias, ab_bias, cp_bias, mp_bias = flags

    nc = bacc.Bacc("TRN2", target_bir_lowering=False, debug=False, num_devices=8)

    dr = {}
    dr["x"] = nc.dram_tensor("x", [T, C], F32, kind="ExternalInput")
    dr["x_img_feats"] = nc.dram_tensor("x_img_feats", [TI, C], F32, kind="ExternalInput")
    for nm, shp in [
        ("b_attn", [3 * C]), ("b_aproj", [C]),
        ("bq", [C]), ("bk", [C]), ("bv", [C]), ("bcproj", [C]),
        ("b_fc", [FF]), ("b_mproj", [C]),
    ]:
        dr[nm] = nc.dram_tensor(nm, shp, F32, kind="ExternalInput")
    for nm, (K, M, Mc) in WPACK.items():
        dr[nm] = nc.dram_tensor(nm, [M // Mc, P, (K // 256) * 2 * Mc], F8,
                                kind="ExternalInput")
    for nm, (K, M) in BPACK.items():
        dr[nm] = nc.dram_tensor(nm, [M // P, P, (K // P) * P], BF16,
                                kind="ExternalInput")
    out_d = nc.dram_tensor("out", [T, C], F32, kind="ExternalOutput")

    with tile.TileContext(nc) as tc, nc.allow_low_precision(
        reason="fp8 DoubleRow projections + bf16 attention are intentional"
    ):
        kw_cms = []

        def openp(**kw):
            cm = tc.tile_pool(**kw)
            return cm, cm.__enter__()

        def openkw(**kw):
            cm, p = openp(**kw)
            kw_cms.append(cm)
            return p

        # ---------------- kernel-wide pools (left-stack base) ----------------
        constp = openkw(name="const", bufs=1)
        fsrp = openkw(name="fsr", bufs=2)       # f32r [128,512] squares
        abp = openkw(name="ab", bufs=1)         # A_b [128,1024] + A_col
        rowp = openkw(name="rows", bufs=4)      # [1,1024] rows
        rbp = openkw(name="rb", bufs=3)         # [64,512] + [1,512] rden

        # ---------------- constants ----------------
        ident = constp.tile([P, P], F32)
        make_identity(nc, ident)
        identR = constp.tile([P, P], F32R)
        nc.vector.tensor_copy(out=identR, in_=ident)

        ones_col = constp.tile([P, 16], F32)
        nc.vector.memset(ones_col, 1.0)
        ones128R = constp.tile([P, 1], F32R)
        nc.vector.tensor_copy(out=ones128R, in_=ones_col[:, 0:1])
        o1x = constp.tile([1, P], F32)
        nc.vector.memset(o1x, 1.0)
        ones_1x128 = constp.tile([1, P], F32R)
        nc.vector.tensor_copy(out=ones_1x128, in_=o1x)
        epsS_t = constp.tile([1, 1], F32)
        nc.vector.memset(epsS_t, EPS * WS * WS)
        zeros384 = constp.tile([P, 384], F32)
        nc.vector.memset(zeros384, 0.0)

        master = constp.tile([P, 896], F32)
        nc.gpsimd.memset(master, 1.0)
        nc.gpsimd.affine_select(
            out=master, in_=master, compare_op=OP.is_ge, fill=0.0,
            base=-384, pattern=[[1, 896]], channel_multiplier=-1)

        # ================= P0: load & transpose x (issued first) =============
        res_cm, residp = openp(name="resid", bufs=NCT, side="right")
        resid = [residp.tile([P, T], F32R, tag="res", name="res") for _ in range(NCT)]

        tok_cm, tokp = openp(name="tok0", bufs=4)
        tp_cm, tpp = openp(name="psT0", bufs=2, space="PSUM")
        toks = []
        for tt in range(NTT):
            tok = tokp.tile([P, C], F32, tag="tok", name="tok")
            nc.sync.dma_start(out=tok, in_=dr["x"].ap()[tt * P:(tt + 1) * P, :])
            toks.append(tok)
        for tt in range(NTT):
            tok = toks[tt]
            for c in range(NCT):
                tps = tpp.tile([P, P], F32, tag="tp", name="tp")
                nc.tensor.transpose(tps, tok[:, c * P:(c + 1) * P], ident)
                if c % 2:
                    nc.vector.tensor_copy(out=resid[c][:, tt * P:(tt + 1) * P], in_=tps)
                else:
                    nc.scalar.copy(out=resid[c][:, tt * P:(tt + 1) * P], in_=tps)
        tp_cm.__exit__(None, None, None)
        tok_cm.__exit__(None, None, None)

        # ---------------- small input rows (issued after x) ----------------
        def load_cols(name, nf):
            t = constp.tile([P, nf], F32, name=name + "_c")
            nc.sync.dma_start(out=t, in_=dr[name].ap().rearrange("(f p) -> p f", p=P))
            return t

        bqk = constp.tile([P, 16], F32)
        nc.sync.dma_start(out=bqk, in_=dr["b_attn"].ap()[0:2 * C].rearrange("(f p) -> p f", p=P))
        bq_c = load_cols("bq", NCT)
        bk_c = load_cols("bk", NCT)
        bap_c = load_cols("b_aproj", NCT)
        bcp_c = load_cols("bcproj", NCT)
        bmp_c = load_cols("b_mproj", NCT)
        bfc_c = load_cols("b_fc", FF // 128)

        # ---------------- helpers ----------------
        def load_wp(name, mc, wpool):
            K, M, Mc = WPACK[name]
            KK = K // 256
            t = wpool.tile([P, KK, 2, Mc], F8, tag="wp", name="wp")
            nc.sync.dma_start(
                out=t,
                in_=dr[name].ap()[mc].rearrange("p (kk ko m) -> p kk ko m", kk=KK, ko=2))
            return t

        def load_wb(name, mc, wpool):
            K, M = BPACK[name]
            nk = K // P
            t = wpool.tile([P, nk, P], BF16, tag="wb", name="wb")
            src = dr[name].ap()[mc].rearrange("p (c m) -> p c m", m=P)
            nc.sync.dma_start(out=t[:, 0:nk // 2, :], in_=src[:, 0:nk // 2, :])
            nc.sync.dma_start(out=t[:, nk // 2:nk, :], in_=src[:, nk // 2:nk, :])
            return t

        def bcast_row(row_f32, dest_pool, psp, tag):
            rowr = rowp.tile([1, C], F32R, tag="row", name="rowr")
            nc.vector.tensor_copy(out=rowr, in_=row_f32)
            dest = dest_pool.tile([P, C], F32, tag=tag, name=tag)
            for cc in range(2):
                bps = psp.tile([P, 512], F32, tag="bc", name="bc")
                nc.tensor.matmul(bps, ones_1x128, rowr[:, 512 * cc:512 * (cc + 1)],
                                 start=True, stop=True)
                nc.scalar.copy(out=dest[:, 512 * cc:512 * (cc + 1)], in_=bps)
            return dest

        def ln_stats(xtiles, psp, with_col=False):
            """A_b [128,T] broadcast of A[t] = 1/(128*std[t]); opt A_col [128,NTT]."""
            sum_ps, sq_ps = [], []
            for tch in range(2):
                sp = psp.tile([1, 512], F32, tag="lnsum", name="lnsum")
                qp = psp.tile([1, 512], F32, tag="lnsq", name="lnsq")
                for c in range(NCT):
                    xs = xtiles[c][:, 512 * tch:512 * (tch + 1)]
                    nc.tensor.matmul(sp, ones128R, xs, start=(c == 0), stop=(c == NCT - 1))
                    sq = fsrp.tile([P, 512], F32R, tag="sq", name="sq")
                    nc.gpsimd.tensor_tensor(out=sq, in0=xs, in1=xs, op=OP.mult)
                    nc.tensor.matmul(qp, ones128R, sq, start=(c == 0), stop=(c == NCT - 1))
                sum_ps.append(sp)
                sq_ps.append(qp)
            mu = rowp.tile([1, T], F32, tag="row", name="mu")
            for tch in range(2):
                nc.vector.tensor_scalar_mul(out=mu[:, 512 * tch:512 * (tch + 1)],
                                            in0=sum_ps[tch], scalar1=1.0 / C)
            musq = rowp.tile([1, T], F32, tag="row", name="musq")
            nc.vector.tensor_tensor(out=musq, in0=mu, in1=mu, op=OP.mult)
            msq = rowp.tile([1, T], F32, tag="row", name="msq")
            for tch in range(2):
                sl = slice(512 * tch, 512 * (tch + 1))
                nc.vector.scalar_tensor_tensor(
                    out=msq[:, sl], in0=sq_ps[tch], scalar=1.0 / C,
                    in1=musq[:, sl], op0=OP.mult, op1=OP.subtract)
            nc.scalar.activation(out=musq, in_=msq, func=AF.Sqrt, bias=epsS_t,
                                 scale=WS * WS)
            arow = rowp.tile([1, T], F32R, tag="row", name="arow")
            nc.vector.reciprocal(out=arow, in_=musq)
            A_b = abp.tile([P, T], F32, tag="A_b", name="A_b")
            for tch in range(2):
                sl = slice(512 * tch, 512 * (tch + 1))
                bps = psp.tile([P, 512], F32, tag="bc", name="bc")
                nc.tensor.matmul(bps, ones_1x128, arow[:, sl], start=True, stop=True)
                nc.scalar.copy(out=A_b[:, sl], in_=bps)
            if not with_col:
                return A_b, None
            A_col = abp.tile([P, NTT], F32, tag="A_col", name="A_col")
            for tt in range(NTT):
                cps = psp.tile([P, P], F32, tag="bc", name="bc")
                nc.tensor.transpose(cps, A_b[:, tt * P:(tt + 1) * P], ident)
                nc.vector.tensor_copy(out=A_col[:, tt:tt + 1], in_=cps[:, 0:1])
            return A_b, A_col

        def quant_x(xtiles, x8, xd8):
            """fp8 copy of the residual stream (+ optional fp8 delta)."""
            for c in range(NCT):
                if c % 2:
                    nc.vector.tensor_copy(out=x8[:, c, :], in_=xtiles[c])
                else:
                    nc.scalar.copy(out=x8[:, c, :], in_=xtiles[c])
            if xd8 is None:
                return
            for c in range(NCT):
                eng = nc.vector if c % 2 else nc.gpsimd
                eng.scalar_tensor_tensor(
                    out=xd8[:, c, :], in0=x8[:, c, :], scalar=-1.0,
                    in1=xtiles[c], op0=OP.mult, op1=OP.add)

        def attn_chunk(kq_of, vaug_tiles, n_s, h, tch, psp, ppool, causal,
                       o_all, o_dt):
            (kt, ko), (qt, qo) = kq_of(h)
            tsl = slice(512 * tch, 512 * (tch + 1))
            ptiles = []
            pair_ps = []
            for pr in range(n_s // 2):
                sps = psp.tile([P, 1024], F32, tag="s", name="s")
                for hf in range(2):
                    st = 2 * pr + hf
                    nc.tensor.matmul(sps[:, 512 * hf:512 * hf + 512],
                                     kt[ko:ko + D, st * P:(st + 1) * P],
                                     qt[qo:qo + D, tsl], start=True, stop=True,
                                     tile_position=(ko, 0))
                pair_ps.append(sps)
            for pr in range(n_s // 2):
                sps = pair_ps[pr]
                pt = ppool.tile([P, 1024], F32R, tag="p", name="p")
                j0 = 2 * pr - 4 * tch
                j1 = j0 + 1
                d0 = causal and j0 >= 0
                d1 = causal and j1 >= 0
                z0 = P * j0 if d0 else 0
                z1 = P * j1 if d1 else 0
                nc.scalar.activation(out=pt[:, z0:1024], in_=sps[:, z0:1024],
                                     func=AF.Exp, scale=0.125)
                if d0 and z0:
                    nc.gpsimd.tensor_copy(out=pt[:, 0:z0], in_=zeros384[:, 0:z0])
                if d1 and z1:
                    nc.gpsimd.tensor_copy(out=pt[:, 512:512 + z1], in_=zeros384[:, 0:z1])
                if d0:
                    nc.gpsimd.tensor_tensor(out=pt[:, z0:z0 + P], in0=pt[:, z0:z0 + P],
                                            in1=master[:, 384:512], op=OP.mult)
                if d1:
                    nc.gpsimd.tensor_tensor(out=pt[:, 512 + z1:512 + z1 + P],
                                            in0=pt[:, 512 + z1:512 + z1 + P],
                                            in1=master[:, 384:512], op=OP.mult)
                ptiles.append(pt)
            ops = psp.tile([65, 512], F32, tag="o", name="o")
            for st in range(n_s):
                pt = ptiles[st // 2][:, 512 * (st % 2):512 * (st % 2) + 512]
                nc.tensor.matmul(ops, vaug_tiles[st][:, 65 * h:65 * h + 65],
                                 pt, start=(st == 0), stop=(st == n_s - 1))
            rden = rbp.tile([1, 512], F32R, tag="rden", name="rden")
            nc.vector.reciprocal(out=rden, in_=ops[64:65, :])
            bps = psp.tile([64, 512], F32, tag="b", name="b")
            nc.tensor.matmul(bps, ones_1x128[:, 0:64], rden, start=True, stop=True)
            rb = rbp.tile([64, 512], F32, tag="rb", name="rb")
            nc.scalar.copy(out=rb, in_=bps)
            po = (h % 2) * D
            nc.vector.tensor_tensor(out=o_all[po:po + D, h // 2, tsl],
                                    in0=ops[0:64, :], in1=rb, op=OP.mult)

        def dr_group(psum, pairs):
            n = len(pairs)
            for i, (lh, rh) in enumerate(pairs):
                nc.tensor.matmul(psum, lh, rh, start=(i == 0), stop=(i == n - 1),
                                 perf_mode=DR)

        def ws_passes(wt, wtR, h8, hd, msl, tsl2):
            ps = [(wt[:, kk, :, msl], h8[:, 2 * kk:2 * kk + 2, tsl2]) for kk in range(4)]
            if hd is not None:
                ps += [(wt[:, kk, :, msl], hd[:, 2 * kk:2 * kk + 2, tsl2]) for kk in range(4)]
            if wtR is not None:
                ps += [(wtR[:, kk, :, msl], h8[:, 2 * kk:2 * kk + 2, tsl2]) for kk in range(4)]
            return ps

        # ================= P1: LN1 + qkv projections =================
        x8_cm, x8p = openp(name="x8", bufs=1)
        x8 = x8p.tile([P, NCT, T], F8, tag="x8", name="x8")
        xd8 = x8p.tile([P, NCT, T], F8, tag="xd8", name="xd8")

        ln_cm, lnp = openp(name="psLN0", bufs=2, space="PSUM")
        A_b, A_col = ln_stats(resid, lnp, with_col=True)
        ln_cm.__exit__(None, None, None)
        quant_x(resid, x8, xd8)

        vap_cm, vap = openp(name="vaug", bufs=NTT, side="right")
        vaug = [vap.tile([P, 16 * 65], F32R, tag="va", name="va") for _ in range(NTT)]

        wv_cm, wv = openp(name="wv", bufs=4)
        accv_cm, accv = openp(name="psACv", bufs=2, space="PSUM")
        brow_v = rowp.tile([1, C], F32, tag="row", name="braw")
        nc.sync.dma_start(out=brow_v,
                          in_=dr["b_attn"].ap()[2 * C:3 * C].rearrange("(a c) -> a c", a=1))
        bvb1 = bcast_row(brow_v, wv, accv, "bvb")
        for cc in range(4):   # v output chunks of 256 cols (4 heads each)
            wt = load_wp("attn_p", 8 + cc, wv)
            wtR = load_wp("vR_p", cc, wv)
            for tt in range(NTT):
                vps = accv.tile([P, 256], F32, tag="acc", name="acc")
                tsl = slice(tt * P, (tt + 1) * P)
                ps = ([(x8[:, 2 * kk:2 * kk + 2, tsl], wt[:, kk, :, :]) for kk in range(4)]
                      + [(xd8[:, 2 * kk:2 * kk + 2, tsl], wt[:, kk, :, :]) for kk in range(4)]
                      + [(x8[:, 2 * kk:2 * kk + 2, tsl], wtR[:, kk, :, :]) for kk in range(4)])
                dr_group(vps, ps)
                dst = vaug[tt].rearrange("p (h x) -> p h x", x=65)[:, 4 * cc:4 * (cc + 1), 0:64]
                nc.vector.scalar_tensor_tensor(
                    out=dst, in0=vps.rearrange("p (h x) -> p h x", x=64),
                    scalar=A_col[:, tt:tt + 1],
                    in1=bvb1[:, 256 * cc:256 * (cc + 1)].rearrange("p (h x) -> p h x", x=64),
                    op0=OP.mult, op1=OP.add)
        for tt in range(NTT):
            nc.vector.tensor_copy(
                out=vaug[tt].rearrange("p (h x) -> p h x", x=65)[:, :, 64:65],
                in_=ones_col.rearrange("p (h x) -> p h x", x=1))

        qk_cm, qkp = openp(name="qk", bufs=16, side="right")
        w1_cm, w1 = openp(name="w1", bufs=3)
        acc_cm, accp = openp(name="psAC1", bufs=4, space="PSUM")
        qk_t = []
        for mc in range(8):
            wt = load_wp("attn_p", mc, w1)
            for mh in range(2):
                f = 2 * mc + mh
                qt = qkp.tile([P, T], BF16, tag="qk", name="qk")
                for tch in range(2):
                    sl = slice(512 * tch, 512 * (tch + 1))
                    aps = accp.tile([P, 512], F32, tag="acc", name="acc")
                    dr_group(aps, ws_passes(wt, None, x8, None,
                                            slice(128 * mh, 128 * mh + 128), sl))
                    nc.vector.tensor_tensor(out=qt[:, sl], in0=aps, in1=A_b[:, sl],
                                            op=OP.mult)
                    if qk_bias:
                        nc.vector.tensor_scalar_add(out=qt[:, sl], in0=qt[:, sl],
                                                    scalar1=bqk[:, f:f + 1])
                qk_t.append(qt)
        acc_cm.__exit__(None, None, None)
        w1_cm.__exit__(None, None, None)
        accv_cm.__exit__(None, None, None)
        wv_cm.__exit__(None, None, None)
        x8_cm.__exit__(None, None, None)

        # ================= P2: self attention =================
        o_cm, opool = openp(name="o1", bufs=1)
        o_all = opool.tile([P, NCT, T], BF16, tag="ot", name="ot")
        pp_cm, pp = openp(name="pp1", bufs=5)
        psS_cm, psS = openp(name="psS1", bufs=2, space="PSUM")

        def kq_self(h):
            return (qk_t[8 + h // 2], (h % 2) * D), (qk_t[h // 2], (h % 2) * D)

        for tch in range(2):
            for h in range(H):
                attn_chunk(kq_self, vaug, 4 * (tch + 1), h, tch, psS, pp,
                           causal=True, o_all=o_all, o_dt=BF16)

        psS_cm.__exit__(None, None, None)
        pp_cm.__exit__(None, None, None)
        qk_cm.__exit__(None, None, None)
        vap_cm.__exit__(None, None, None)

        # ================= P3: aproj (bf16) + residual in place ======
        w2_cm, w2 = openp(name="w2", bufs=3)
        acc_cm, accp = openp(name="psAC3", bufs=3, space="PSUM")
        for co in range(NCT):
            wt = load_wb("aproj_b", co, w2)
            for tch in range(2):
                sl = slice(512 * tch, 512 * (tch + 1))
                aps = accp.tile([P, 512], F32, tag="acc", name="acc")
                for c in range(NCT):
                    nc.tensor.matmul(aps, wt[:, c, :], o_all[:, c, sl],
                                     start=(c == 0), stop=(c == NCT - 1))
                nc.vector.tensor_tensor(out=resid[co][:, sl], in0=aps,
                                        in1=resid[co][:, sl], op=OP.add)
                if ab_bias:
                    nc.vector.tensor_scalar_add(
                        out=resid[co][:, sl], in0=resid[co][:, sl],
                        scalar1=bap_c[:, co:co + 1])
        acc_cm.__exit__(None, None, None)
        w2_cm.__exit__(None, None, None)
        o_cm.__exit__(None, None, None)

        # ================= P4: cross attention projections =================
        x1_cm, x1p = openp(name="x18", bufs=1)
        x18 = x1p.tile([P, NCT, T], F8, tag="x8", name="x8")

        ln_cm, lnp = openp(name="psLN1", bufs=2, space="PSUM")
        A_b, _ = ln_stats(resid, lnp)
        ln_cm.__exit__(None, None, None)
        quant_x(resid, x18, None)

        k2_cm, k2p = openp(name="k2", bufs=NCT, side="right")
        v2_cm, v2p = openp(name="v2", bufs=2, side="right")

        w3_cm, w3 = openp(name="w3", bufs=3)

        img_cm, imgp = openp(name="img", bufs=1)
        tok_cm, tokp = openp(name="tok4", bufs=2)
        tp_cm, tpp = openp(name="psT4", bufs=2, space="PSUM")
        imgT = imgp.tile([P, NCT, TI], F8, tag="imgT", name="imgT")
        for tt in range(TI // P):
            tok = tokp.tile([P, C], F32, tag="tok", name="tok")
            nc.sync.dma_start(out=tok, in_=dr["x_img_feats"].ap()[tt * P:(tt + 1) * P, :])
            for c in range(NCT):
                tps = tpp.tile([P, P], F32, tag="tp", name="tp")
                nc.tensor.transpose(tps, tok[:, c * P:(c + 1) * P], ident)
                nc.vector.tensor_copy(out=imgT[:, c, tt * P:(tt + 1) * P], in_=tps)
        tp_cm.__exit__(None, None, None)
        tok_cm.__exit__(None, None, None)

        acc_cm, accp = openp(name="psAC4", bufs=2, space="PSUM")
        k2_t = []
        for mc in range(4):
            wt = load_wp("k_p", mc, w3)
            for mh in range(2):
                f = 2 * mc + mh
                kt = k2p.tile([P, TI], BF16, tag="k2", name="k2")
                kps = accp.tile([P, 256], F32, tag="acc256", name="acc256")
                dr_group(kps, [(wt[:, kk, :, 128 * mh:128 * mh + 128],
                                imgT[:, 2 * kk:2 * kk + 2, :]) for kk in range(4)])
                nc.scalar.activation(out=kt, in_=kps, func=AF.Identity,
                                     bias=bk_c[:, f:f + 1], scale=WSI)
                k2_t.append(kt)

        brow_v2 = rowp.tile([1, C], F32, tag="row", name="braw2")
        nc.sync.dma_start(out=brow_v2, in_=dr["bv"].ap().rearrange("(a c) -> a c", a=1))
        wv2_cm, wv2 = openp(name="wv2", bufs=3)
        bvb2 = bcast_row(brow_v2, wv2, accp, "bvb2")

        v2aug = [v2p.tile([P, 16 * 65], F32R, tag="va2", name="va2")
                 for _ in range(TI // P)]
        for cc in range(4):
            wt = load_wp("v2_p", cc, wv2)
            for st in range(TI // P):
                vps = accp.tile([P, 256], F32, tag="acc256", name="acc256")
                dr_group(vps, [(imgT[:, 2 * kk:2 * kk + 2, st * P:(st + 1) * P],
                                wt[:, kk, :, :]) for kk in range(4)])
                dst = v2aug[st].rearrange("p (h x) -> p h x", x=65)[:, 4 * cc:4 * (cc + 1), 0:64]
                nc.vector.scalar_tensor_tensor(
                    out=dst, in0=vps.rearrange("p (h x) -> p h x", x=64),
                    scalar=WSI,
                    in1=bvb2[:, 256 * cc:256 * (cc + 1)].rearrange("p (h x) -> p h x", x=64),
                    op0=OP.mult, op1=OP.add)
        for st in range(TI // P):
            nc.vector.tensor_copy(
                out=v2aug[st].rearrange("p (h x) -> p h x", x=65)[:, :, 64:65],
                in_=ones_col.rearrange("p (h x) -> p h x", x=1))
        wv2_cm.__exit__(None, None, None)
        img_cm.__exit__(None, None, None)

        q2_cm, q2p = openp(name="q2", bufs=NCT, side="right")
        q2_t = []
        for mc in range(4):
            wt = load_wp("q_p", mc, w3)
            for mh in range(2):
                f = 2 * mc + mh
                qt = q2p.tile([P, T], BF16, tag="q2", name="q2")
                for tch in range(2):
                    sl = slice(512 * tch, 512 * (tch + 1))
                    aps = accp.tile([P, 512], F32, tag="acc", name="acc")
                    dr_group(aps, ws_passes(wt, None, x18, None,
                                            slice(128 * mh, 128 * mh + 128), sl))
                    nc.vector.tensor_tensor(out=qt[:, sl], in0=aps, in1=A_b[:, sl],
                                            op=OP.mult)
                    if q2_bias:
                        nc.vector.tensor_scalar_add(out=qt[:, sl], in0=qt[:, sl],
                                                    scalar1=bq_c[:, f:f + 1])
                q2_t.append(qt)
        acc_cm.__exit__(None, None, None)
        w3_cm.__exit__(None, None, None)
        x1_cm.__exit__(None, None, None)

        # ================= P5: cross attention =================
        o_cm, opool = openp(name="o2", bufs=1)
        o2_all = opool.tile([P, NCT, T], F8, tag="ot", name="ot")
        pp_cm, pp = openp(name="pp2", bufs=4)
        psS_cm, psS = openp(name="psS2", bufs=2, space="PSUM")

        def kq_cross(h):
            return (k2_t[h // 2], (h % 2) * D), (q2_t[h // 2], (h % 2) * D)

        for tch in range(2):
            for h in range(H):
                attn_chunk(kq_cross, v2aug, TI // P, h, tch, psS, pp,
                           causal=False, o_all=o2_all, o_dt=F8)

        psS_cm.__exit__(None, None, None)
        pp_cm.__exit__(None, None, None)
        q2_cm.__exit__(None, None, None)
        v2_cm.__exit__(None, None, None)
        k2_cm.__exit__(None, None, None)

        # ================= P6: cproj + residual (x2, in place) =================
        w4_cm, w4 = openp(name="w4", bufs=3)
        acc_cm, accp = openp(name="psAC5", bufs=3, space="PSUM")
        for mc in range(4):
            wt = load_wp("cproj_p", mc, w4)
            for mh in range(2):
                co = 2 * mc + mh
                for tch in range(2):
                    sl = slice(512 * tch, 512 * (tch + 1))
                    aps = accp.tile([P, 512], F32, tag="acc", name="acc")
                    dr_group(aps, ws_passes(wt, None, o2_all, None,
                                            slice(128 * mh, 128 * mh + 128), sl))
                    nc.vector.scalar_tensor_tensor(
                        out=resid[co][:, sl], in0=aps, scalar=WSI,
                        in1=resid[co][:, sl], op0=OP.mult, op1=OP.add)
                    if cp_bias:
                        nc.vector.tensor_scalar_add(
                            out=resid[co][:, sl], in0=resid[co][:, sl],
                            scalar1=bcp_c[:, co:co + 1])
        acc_cm.__exit__(None, None, None)
        w4_cm.__exit__(None, None, None)
        o_cm.__exit__(None, None, None)

        # ================= P7: MLP =================
        x2_cm, x2p = openp(name="x28", bufs=1)
        x28 = x2p.tile([P, NCT, T], F8, tag="x8", name="x8")
        x2d8 = x2p.tile([P, NCT, T], F8, tag="xd8", name="xd8")

        ln_cm, lnp = openp(name="psLN2", bufs=2, space="PSUM")
        A_b, _ = ln_stats(resid, lnp)
        ln_cm.__exit__(None, None, None)
        quant_x(resid, x28, x2d8)

        up_cm, up = openp(name="u", bufs=16, side="right")
        utiles = [up.tile([P, 2, T], BF16, tag="u", name="u") for _ in range(16)]
        uscr_cm, uscrp = openp(name="uscr", bufs=4)
        w5_cm, w5 = openp(name="w5", bufs=4)
        accU_cm, accU = openp(name="psU", bufs=4, space="PSUM")
        for mc in range(16):
            wt = load_wp("fc_p", mc, w5)
            wtR = load_wp("fcR_p", mc, w5)
            for mh in range(2):
                ff = 2 * mc + mh
                for tch in range(2):
                    sl = slice(512 * tch, 512 * (tch + 1))
                    ups = accU.tile([P, 512], F32, tag="acc", name="acc")
                    dr_group(ups, ws_passes(wt, wtR, x28, x2d8,
                                            slice(128 * mh, 128 * mh + 128), sl))
                    uscr = uscrp.tile([P, 512], F32, tag="us", name="us")
                    nc.vector.tensor_tensor(out=uscr, in0=ups, in1=A_b[:, sl],
                                            op=OP.mult)
                    nc.scalar.activation(out=utiles[ff // 2][:, ff % 2, sl], in_=uscr,
                                         func=AF.Gelu_apprx_tanh,
                                         bias=bfc_c[:, ff:ff + 1], scale=1.0)
        accU_cm.__exit__(None, None, None)
        w5_cm.__exit__(None, None, None)
        uscr_cm.__exit__(None, None, None)
        x2_cm.__exit__(None, None, None)

        w6_cm, w6 = openp(name="w6", bufs=3)
        psM_cm, psM = openp(name="psM", bufs=3, space="PSUM")
        for co in range(NCT):
            wt = load_wb("mproj_b", co, w6)
            for tch in range(2):
                sl = slice(512 * tch, 512 * (tch + 1))
                mps = psM.tile([P, 512], F32, tag="m", name="m")
                for ff in range(FF // P):
                    nc.tensor.matmul(mps, wt[:, ff, :], utiles[ff // 2][:, ff % 2, sl],
                                     start=(ff == 0), stop=(ff == FF // P - 1))
                nc.vector.tensor_tensor(out=resid[co][:, sl], in0=mps,
                                        in1=resid[co][:, sl], op=OP.add)
                if mp_bias:
                    nc.vector.tensor_scalar_add(
                        out=resid[co][:, sl], in0=resid[co][:, sl],
                        scalar1=bmp_c[:, co:co + 1])
        psM_cm.__exit__(None, None, None)
        w6_cm.__exit__(None, None, None)
        up_cm.__exit__(None, None, None)

        # ================= P8: transpose back & store =================
        tok_cm, tokp = openp(name="tok7", bufs=2)
        tp_cm, tpp = openp(name="psT7", bufs=4, space="PSUM")
        for tt in range(NTT):
            otok = tokp.tile([P, C], F32, tag="tok", name="tok")
            for c in range(NCT):
                tps = tpp.tile([P, P], F32R, tag="tpr", name="tpr")
                nc.tensor.transpose(tps, resid[c][:, tt * P:(tt + 1) * P], identR)
                if c % 2:
                    nc.vector.tensor_copy(out=otok[:, c * P:(c + 1) * P], in_=tps)
                else:
                    nc.scalar.copy(out=otok[:, c * P:(c + 1) * P], in_=tps)
            nc.sync.dma_start(out=out_d.ap()[tt * P:(tt + 1) * P, :], in_=otok)
        tp_cm.__exit__(None, None, None)
        tok_cm.__exit__(None, None, None)
        res_cm.__exit__(None, None, None)

        for cm in reversed(kw_cms):
            cm.__exit__(None, None, None)

    nc.compile()
    return nc


def _pack_core(Wq, Mc):
    K, M = Wq.shape
    KK, MC = K // 256, M // Mc
    A = Wq.reshape(KK, 2, P, MC, Mc).transpose(3, 2, 0, 1, 4)
    return np.ascontiguousarray(A.reshape(MC, P, KK * 2 * Mc))


def _pack_w(W, Mc=256):
    import ml_dtypes
    return _pack_core((np.asarray(W, np.float32) * WS).astype(ml_dtypes.float8_e4m3), Mc)


def _pack_wr(W, Mc=256):
    import ml_dtypes
    Ws = np.asarray(W, np.float32) * WS
    W8 = Ws.astype(ml_dtypes.float8_e4m3)
    R = Ws - W8.astype(np.float32)
    return _pack_core(R.astype(ml_dtypes.float8_e4m3), Mc)


def _pack_b(W):
    """[K, M] f32 -> [M//128, 128, (K//128)*128] bf16 stationary slabs."""
    import ml_dtypes
    K, M = W.shape
    A = np.asarray(W, np.float32).reshape(K // P, P, M // P, P).transpose(2, 1, 0, 3)
    return np.ascontiguousarray(A.reshape(M // P, P, K).astype(ml_dtypes.bfloat16))


def _fold(W, g):
    Wg = np.asarray(W, np.float32) * np.asarray(g, np.float32)[:, None]
    return Wg - Wg.mean(0, keepdims=True)


def kernel(**inputs):
    from concourse.bass_utils import run_bass_kernel_spmd

    np_inputs = {k: np.asarray(v, dtype=np.float32) for k, v in inputs.items()}
    g1, b1 = np_inputs["ln1_g"], np_inputs["ln1_b"]
    g2, b2 = np_inputs["ln2_g"], np_inputs["ln2_b"]
    Wa = np_inputs["W_attn"]

    # fold LN gains/means into weights; fold LN biases into effective biases
    Wa_f = _fold(Wa, g1)
    Wq_f = _fold(np_inputs["Wq"], g1)
    Wfc_f = _fold(np_inputs["W_fc"], g2)
    b_attn_eff = np_inputs["b_attn"] + Wa.T @ b1
    bq_eff = np_inputs["bq"] + np_inputs["Wq"].T @ b1
    bfc_eff = np_inputs["b_fc"] + np_inputs["W_fc"].T @ b2

    flags = (bool(np.any(b_attn_eff[0:2 * C])), bool(np.any(bq_eff)),
             bool(np.any(np_inputs["b_aproj"])), bool(np.any(np_inputs["bcproj"])),
             bool(np.any(np_inputs["b_mproj"])))
    key = ("nc", flags)
    if key not in _CACHED:
        _CACHED[key] = _build(flags)
    nc = _CACHED[key]

    packed = {
        "attn_p": _pack_w(Wa_f),
        "vR_p": _pack_wr(Wa_f[:, 2 * C:3 * C]),
        "q_p": _pack_w(Wq_f),
        "k_p": _pack_w(np_inputs["Wk"]),
        "v2_p": _pack_w(np_inputs["Wv"]),
        "cproj_p": _pack_w(np_inputs["Wcproj"]),
        "fc_p": _pack_w(Wfc_f),
        "fcR_p": _pack_wr(Wfc_f),
        "aproj_b": _pack_b(np_inputs["W_aproj"]),
        "mproj_b": _pack_b(np_inputs["W_mproj"]),
    }
    small = {
        "b_attn": b_attn_eff, "bq": bq_eff, "b_fc": bfc_eff,
        "b_aproj": np_inputs["b_aproj"], "bk": np_inputs["bk"],
        "bv": np_inputs["bv"], "bcproj": np_inputs["bcproj"],
        "b_mproj": np_inputs["b_mproj"],
    }
    in_maps = []
    for b in range(B):
        m = dict(small)
        m.update(packed)
        m["x"] = np.ascontiguousarray(np_inputs["x"][b])
        m["x_img_feats"] = np.ascontiguousarray(np_inputs["x_img_feats"][b])
        in_maps.append(m)
    res = run_bass_kernel_spmd(nc, in_maps, core_ids=list(range(B)))
    out = np.stack([res.results[b]["out"] for b in range(B)], axis=0)
    return out.astype(np.float32)


# revision 20
# speedup vs baseline: 1.4337x; 1.0623x over previous
"""Trainium2 Bass kernel for a dense transformer block (self-attn + cross-attn + MLP).

Sharding: data-parallel over batch, one batch element per NeuronCore (B=8, 8 cores),
no collectives. Activations are feature-major ([C, T]) on chip.

LayerNorm is FOLDED into the projection weights on the host:
    W' = g*W - colmean(g*W),  bias' = b + W^T ln_b
so projections consume the RAW residual x (quantized straight off the stream,
no LN-apply pass); the per-token scale A[t] = 1/(128*std[t]) is applied at PSUM
evacuation (a [128,T] broadcast tile built once per LN via a ones-matmul).

Precision plan (validated in a calibrated numpy emulator, rel err ~3.8e-3):
  q,k, cross q2/k2/v2, cproj:  1-pass fp8e4m3 DoubleRow (weights x128 on host)
  v, fc:                       3-pass DoubleRow at one PSUM scale:
                               W8*x8 + W8*dx8 + R8*x8  (~bf16 accuracy, 0.75x
                               bf16 PE cost); dx8 = fp8(x - x8) via subnormals
  aproj, mproj:                bf16 (their inputs o / u are produced bf16
                               directly, avoiding on-chip hi/lo splits)
Attention interior: q/k bf16, exp/P f32r, V-aug f32r with ones column for the
softmax denominator; causal masking via one precomputed [128,896] master mask.
Softmax exp runs on paired PSUM banks ([128,1024] per ACT op); masks, squares
and fp8 deltas run on GPSIMD to keep DVE available for PSUM-coupled work.
The residual stream lives in SBUF for the whole kernel.
"""

import sys
import numpy as np

sys.path.insert(0, "/opt/trn_rl_repo")

B, T, C = 8, 1024, 1024
H = 16
D = C // H          # 64
TI = 256
FF = 4 * C          # 4096
EPS = 1e-5
NCT = C // 128      # 8 c tiles
NTT = T // 128      # 8 t tiles
P = 128
WS = 128.0          # fp8 weight scale
WSI = 1.0 / WS

_CACHED = {}

# fp8 DR packs: [MC, 128, KK*2*Mc], elem [mc, p, (kk, ko, m)] =
# q8(WS*W)[256*kk + 128*ko + p, mc*Mc + m]; *R_p carry fp8(WS*W - deq(W8)).
WPACK = {
    "attn_p": (C, 3 * C, 256),
    "vR_p": (C, C, 256),
    "q_p": (C, C, 256),
    "k_p": (C, C, 256),
    "v2_p": (C, C, 256),
    "cproj_p": (C, C, 256),
    "fc_p": (C, FF, 256),
    "fcR_p": (C, FF, 256),
}
# bf16 stationary packs: [M//128, 128, (K//128)*128], elem [mc, p, (c, m)] =
# bf16(W)[128*c + p, 128*mc + m]
BPACK = {"aproj_b": (C, C), "mproj_b": (FF, C)}


def _build(flags):
    import concourse.tile as tile
    from concourse import bacc, mybir
    from concourse.masks import make_identity

    F32, F32R = mybir.dt.float32, mybir.dt.float32r
    BF16 = mybir.dt.bfloat16
    F8 = mybir.dt.float8e4
    AF = mybir.ActivationFunctionType
    OP = mybir.AluOpType
    DR = mybir.MatmulPerfMode.DoubleRow

    qk_bias, q2_bias, ab_bias, cp_bias, mp_bias = flags

    nc = bacc.Bacc("TRN2", target_bir_lowering=False, debug=False, num_devices=8)

    dr = {}
    dr["x"] = nc.dram_tensor("x", [T, C], F32, kind="ExternalInput")
    dr["x_img_feats"] = nc.dram_tensor("x_img_feats", [TI, C], F32, kind="ExternalInput")
    for nm, shp in [
        ("b_attn", [3 * C]), ("b_aproj", [C]),
        ("bq", [C]), ("bk", [C]), ("bv", [C]), ("bcproj", [C]),
        ("b_fc", [FF]), ("b_mproj", [C]),
    ]:
        dr[nm] = nc.dram_tensor(nm, shp, F32, kind="ExternalInput")
    for nm, (K, M, Mc) in WPACK.items():
        dr[nm] = nc.dram_tensor(nm, [M // Mc, P, (K // 256) * 2 * Mc], F8,
                                kind="ExternalInput")
    for nm, (K, M) in BPACK.items():
        dr[nm] = nc.dram_tensor(nm, [M // P, P, (K // P) * P], BF16,
                                kind="ExternalInput")
    out_d = nc.dram_tensor("out", [T, C], F32, kind="ExternalOutput")

    with tile.TileContext(nc) as tc, nc.allow_low_precision(
        reason="fp8 DoubleRow projections + bf16 attention are intentional"
    ):
        kw_cms = []

        def openp(**kw):
            cm = tc.tile_pool(**kw)
            return cm, cm.__enter__()

        def openkw(**kw):
            cm, p = openp(**kw)
            kw_cms.append(cm)
            return p

        # ---------------- kernel-wide pools (left-stack base) ----------------
        constp = openkw(name="const", bufs=1)
        fsrp = openkw(name="fsr", bufs=2)       # f32r [128,512] squares
        abp = openkw(name="ab", bufs=1)         # A_b [128,1024] + A_col
        rowp = openkw(name="rows", bufs=4)      # [1,1024] rows
        rbp = openkw(name="rb", bufs=3)         # [64,512] + [1,512] rden

        # ---------------- constants ----------------
        ident = constp.tile([P, P], F32)
        make_identity(nc, ident)
        identR = constp.tile([P, P], F32R)
        nc.vector.tensor_copy(out=identR, in_=ident)

        ones_col = constp.tile([P, 16], F32)
        nc.vector.memset(ones_col, 1.0)
        ones128R = constp.tile([P, 1], F32R)
        nc.vector.tensor_copy(out=ones128R, in_=ones_col[:, 0:1])
        o1x = constp.tile([1, P], F32)
        nc.vector.memset(o1x, 1.0)
        ones_1x128 = constp.tile([1, P], F32R)
        nc.vector.tensor_copy(out=ones_1x128, in_=o1x)
        epsS_t = constp.tile([1, 1], F32)
        nc.vector.memset(epsS_t, EPS * WS * WS)
        zeros384 = constp.tile([P, 384], F32)
        nc.vector.memset(zeros384, 0.0)

        master = constp.tile([P, 896], F32)
        nc.gpsimd.memset(master, 1.0)
        nc.gpsimd.affine_select(
            out=master, in_=master, compare_op=OP.is_ge, fill=0.0,
            base=-384, pattern=[[1, 896]], channel_multiplier=-1)

        # ================= P0: load & transpose x (issued first) =============
        res_cm, residp = openp(name="resid", bufs=NCT, side="right")
        resid = [residp.tile([P, T], F32R, tag="res", name="res") for _ in range(NCT)]

        tok_cm, tokp = openp(name="tok0", bufs=4)
        tp_cm, tpp = openp(name="psT0", bufs=2, space="PSUM")
        toks = []
        for tt in range(NTT):
            tok = tokp.tile([P, C], F32, tag="tok", name="tok")
            nc.sync.dma_start(out=tok, in_=dr["x"].ap()[tt * P:(tt + 1) * P, :])
            toks.append(tok)
        for tt in range(NTT):
            tok = toks[tt]
            for c in range(NCT):
                tps = tpp.tile([P, P], F32, tag="tp", name="tp")
                nc.tensor.transpose(tps, tok[:, c * P:(c + 1) * P], ident)
                if c % 2:
                    nc.vector.tensor_copy(out=resid[c][:, tt * P:(tt + 1) * P], in_=tps)
                else:
                    nc.scalar.copy(out=resid[c][:, tt * P:(tt + 1) * P], in_=tps)
        tp_cm.__exit__(None, None, None)
        tok_cm.__exit__(None, None, None)

        # ---------------- small input rows (issued after x) ----------------
        def load_cols(name, nf):
            t = constp.tile([P, nf], F32, name=name + "_c")
            nc.sync.dma_start(out=t, in_=dr[name].ap().rearrange("(f p) -> p f", p=P))
            return t

        bqk = constp.tile([P, 16], F32)
        nc.sync.dma_start(out=bqk, in_=dr["b_attn"].ap()[0:2 * C].rearrange("(f p) -> p f", p=P))
        bq_c = load_cols("bq", NCT)
        bk_c = load_cols("bk", NCT)
        bap_c = load_cols("b_aproj", NCT)
        bcp_c = load_cols("bcproj", NCT)
        bmp_c = load_cols("b_mproj", NCT)
        bfc_c = load_cols("b_fc", FF // 128)

        # ---------------- helpers ----------------
        def load_wp(name, mc, wpool):
            K, M, Mc = WPACK[name]
            KK = K // 256
            t = wpool.tile([P, KK, 2, Mc], F8, tag="wp", name="wp")
            nc.sync.dma_start(
                out=t,
                in_=dr[name].ap()[mc].rearrange("p (kk ko m) -> p kk ko m", kk=KK, ko=2))
            return t

        def load_wb(name, mc, wpool):
            K, M = BPACK[name]
            nk = K // P
            t = wpool.tile([P, nk, P], BF16, tag="wb", name="wb")
            src = dr[name].ap()[mc].rearrange("p (c m) -> p c m", m=P)
            nc.sync.dma_start(out=t[:, 0:nk // 2, :], in_=src[:, 0:nk // 2, :])
            nc.sync.dma_start(out=t[:, nk // 2:nk, :], in_=src[:, nk // 2:nk, :])
            return t

        def bcast_row(row_f32, dest_pool, psp, tag):
            rowr = rowp.tile([1, C], F32R, tag="row", name="rowr")
            nc.vector.tensor_copy(out=rowr, in_=row_f32)
            dest = dest_pool.tile([P, C], F32, tag=tag, name=tag)
            for cc in range(2):
                bps = psp.tile([P, 512], F32, tag="bc", name="bc")
                nc.tensor.matmul(bps, ones_1x128, rowr[:, 512 * cc:512 * (cc + 1)],
                                 start=True, stop=True)
                nc.scalar.copy(out=dest[:, 512 * cc:512 * (cc + 1)], in_=bps)
            return dest

        def ln_stats(xtiles, psp, with_col=False):
            """A_b [128,T] broadcast of A[t] = 1/(128*std[t]); opt A_col [128,NTT]."""
            sum_ps, sq_ps = [], []
            for tch in range(2):
                sp = psp.tile([1, 512], F32, tag="lnsum", name="lnsum")
                qp = psp.tile([1, 512], F32, tag="lnsq", name="lnsq")
                for c in range(NCT):
                    xs = xtiles[c][:, 512 * tch:512 * (tch + 1)]
                    nc.tensor.matmul(sp, ones128R, xs, start=(c == 0), stop=(c == NCT - 1))
                    sq = fsrp.tile([P, 512], F32R, tag="sq", name="sq")
                    nc.scalar.activation(out=sq, in_=xs, func=AF.Square, scale=1.0)
                    nc.tensor.matmul(qp, ones128R, sq, start=(c == 0), stop=(c == NCT - 1))
                sum_ps.append(sp)
                sq_ps.append(qp)
            mu = rowp.tile([1, T], F32, tag="row", name="mu")
            for tch in range(2):
                nc.vector.tensor_scalar_mul(out=mu[:, 512 * tch:512 * (tch + 1)],
                                            in0=sum_ps[tch], scalar1=1.0 / C)
            musq = rowp.tile([1, T], F32, tag="row", name="musq")
            nc.vector.tensor_tensor(out=musq, in0=mu, in1=mu, op=OP.mult)
            msq = rowp.tile([1, T], F32, tag="row", name="msq")
            for tch in range(2):
                sl = slice(512 * tch, 512 * (tch + 1))
                nc.vector.scalar_tensor_tensor(
                    out=msq[:, sl], in0=sq_ps[tch], scalar=1.0 / C,
                    in1=musq[:, sl], op0=OP.mult, op1=OP.subtract)
            nc.scalar.activation(out=musq, in_=msq, func=AF.Sqrt, bias=epsS_t,
                                 scale=WS * WS)
            arow = rowp.tile([1, T], F32R, tag="row", name="arow")
            nc.vector.reciprocal(out=arow, in_=musq)
            A_b = abp.tile([P, T], F32, tag="A_b", name="A_b")
            for tch in range(2):
                sl = slice(512 * tch, 512 * (tch + 1))
                bps = psp.tile([P, 512], F32, tag="bc", name="bc")
                nc.tensor.matmul(bps, ones_1x128, arow[:, sl], start=True, stop=True)
                nc.scalar.copy(out=A_b[:, sl], in_=bps)
            if not with_col:
                return A_b, None
            A_col = abp.tile([P, NTT], F32, tag="A_col", name="A_col")
            for tt in range(NTT):
                cps = psp.tile([P, P], F32, tag="bc", name="bc")
                nc.tensor.transpose(cps, A_b[:, tt * P:(tt + 1) * P], ident)
                nc.vector.tensor_copy(out=A_col[:, tt:tt + 1], in_=cps[:, 0:1])
            return A_b, A_col

        def quant_x(xtiles, x8, xd8):
            """fp8 copy of the residual stream (+ optional fp8 delta)."""
            for c in range(NCT):
                eng = nc.vector if c % 2 else nc.gpsimd
                eng.tensor_copy(out=x8[:, c, :], in_=xtiles[c])
            if xd8 is None:
                return
            for c in range(NCT):
                eng = nc.vector if c % 2 else nc.gpsimd
                eng.scalar_tensor_tensor(
                    out=xd8[:, c, :], in0=x8[:, c, :], scalar=-1.0,
                    in1=xtiles[c], op0=OP.mult, op1=OP.add)

        def attn_chunk(kq_of, vaug_tiles, n_s, h, tch, psp, ppool, causal,
                       o_all, o_dt):
            (kt, ko), (qt, qo) = kq_of(h)
            tsl = slice(512 * tch, 512 * (tch + 1))
            ptiles = []
            pair_ps = []
            for pr in range(n_s // 2):
                sps = psp.tile([P, 1024], F32, tag="s", name="s")
                for hf in range(2):
                    st = 2 * pr + hf
                    nc.tensor.matmul(sps[:, 512 * hf:512 * hf + 512],
                                     kt[ko:ko + D, st * P:(st + 1) * P],
                                     qt[qo:qo + D, tsl], start=True, stop=True,
                                     tile_position=(ko, 0))
                pair_ps.append(sps)
            for pr in range(n_s // 2):
                sps = pair_ps[pr]
                pt = ppool.tile([P, 1024], F32R, tag="p", name="p")
                j0 = 2 * pr - 4 * tch
                j1 = j0 + 1
                d0 = causal and j0 >= 0
                d1 = causal and j1 >= 0
                z0 = P * j0 if d0 else 0
                z1 = P * j1 if d1 else 0
                nc.scalar.activation(out=pt[:, z0:1024], in_=sps[:, z0:1024],
                                     func=AF.Exp, scale=0.125)
                if d0 and z0:
                    nc.gpsimd.tensor_copy(out=pt[:, 0:z0], in_=zeros384[:, 0:z0])
                if d1 and z1:
                    nc.gpsimd.tensor_copy(out=pt[:, 512:512 + z1], in_=zeros384[:, 0:z1])
                if d0:
                    nc.gpsimd.tensor_tensor(out=pt[:, z0:z0 + P], in0=pt[:, z0:z0 + P],
                                            in1=master[:, 384:512], op=OP.mult)
                if d1:
                    nc.gpsimd.tensor_tensor(out=pt[:, 512 + z1:512 + z1 + P],
                                            in0=pt[:, 512 + z1:512 + z1 + P],
                                            in1=master[:, 384:512], op=OP.mult)
                ptiles.append(pt)
            ops = psp.tile([65, 512], F32, tag="o", name="o")
            for st in range(n_s):
                pt = ptiles[st // 2][:, 512 * (st % 2):512 * (st % 2) + 512]
                nc.tensor.matmul(ops, vaug_tiles[st][:, 65 * h:65 * h + 65],
                                 pt, start=(st == 0), stop=(st == n_s - 1))
            rden = rbp.tile([1, 512], F32R, tag="rden", name="rden")
            nc.vector.reciprocal(out=rden, in_=ops[64:65, :])
            bps = psp.tile([64, 512], F32, tag="b", name="b")
            nc.tensor.matmul(bps, ones_1x128[:, 0:64], rden, start=True, stop=True)
            rb = rbp.tile([64, 512], F32, tag="rb", name="rb")
            if h % 2:
                nc.vector.tensor_copy(out=rb, in_=bps)
            else:
                nc.scalar.copy(out=rb, in_=bps)
            po = (h % 2) * D
            nc.vector.tensor_tensor(out=o_all[po:po + D, h // 2, tsl],
                                    in0=ops[0:64, :], in1=rb, op=OP.mult)

        def dr_group(psum, pairs):
            n = len(pairs)
            for i, (lh, rh) in enumerate(pairs):
                nc.tensor.matmul(psum, lh, rh, start=(i == 0), stop=(i == n - 1),
                                 perf_mode=DR)

        def ws_passes(wt, wtR, h8, hd, msl, tsl2):
            ps = [(wt[:, kk, :, msl], h8[:, 2 * kk:2 * kk + 2, tsl2]) for kk in range(4)]
            if hd is not None:
                ps += [(wt[:, kk, :, msl], hd[:, 2 * kk:2 * kk + 2, tsl2]) for kk in range(4)]
            if wtR is not None:
                ps += [(wtR[:, kk, :, msl], h8[:, 2 * kk:2 * kk + 2, tsl2]) for kk in range(4)]
            return ps

        # ================= P1: LN1 + qkv projections =================
        x8_cm, x8p = openp(name="x8", bufs=1)
        x8 = x8p.tile([P, NCT, T], F8, tag="x8", name="x8")
        xd8 = x8p.tile([P, NCT, T], F8, tag="xd8", name="xd8")

        ln_cm, lnp = openp(name="psLN0", bufs=2, space="PSUM")
        A_b, A_col = ln_stats(resid, lnp, with_col=True)
        ln_cm.__exit__(None, None, None)
        quant_x(resid, x8, xd8)

        vap_cm, vap = openp(name="vaug", bufs=NTT, side="right")
        vaug = [vap.tile([P, 16 * 65], F32R, tag="va", name="va") for _ in range(NTT)]

        wv_cm, wv = openp(name="wv", bufs=4)
        accv_cm, accv = openp(name="psACv", bufs=2, space="PSUM")
        brow_v = rowp.tile([1, C], F32, tag="row", name="braw")
        nc.sync.dma_start(out=brow_v,
                          in_=dr["b_attn"].ap()[2 * C:3 * C].rearrange("(a c) -> a c", a=1))
        bvb1 = bcast_row(brow_v, wv, accv, "bvb")
        for cc in range(4):   # v output chunks of 256 cols (4 heads each)
            wt = load_wp("attn_p", 8 + cc, wv)
            wtR = load_wp("vR_p", cc, wv)
            for tt in range(NTT):
                vps = accv.tile([P, 256], F32, tag="acc", name="acc")
                tsl = slice(tt * P, (tt + 1) * P)
                ps = ([(x8[:, 2 * kk:2 * kk + 2, tsl], wt[:, kk, :, :]) for kk in range(4)]
                      + [(xd8[:, 2 * kk:2 * kk + 2, tsl], wt[:, kk, :, :]) for kk in range(4)]
                      + [(x8[:, 2 * kk:2 * kk + 2, tsl], wtR[:, kk, :, :]) for kk in range(4)])
                dr_group(vps, ps)
                dst = vaug[tt].rearrange("p (h x) -> p h x", x=65)[:, 4 * cc:4 * (cc + 1), 0:64]
                nc.vector.scalar_tensor_tensor(
                    out=dst, in0=vps.rearrange("p (h x) -> p h x", x=64),
                    scalar=A_col[:, tt:tt + 1],
                    in1=bvb1[:, 256 * cc:256 * (cc + 1)].rearrange("p (h x) -> p h x", x=64),
                    op0=OP.mult, op1=OP.add)
        for tt in range(NTT):
            nc.vector.tensor_copy(
                out=vaug[tt].rearrange("p (h x) -> p h x", x=65)[:, :, 64:65],
                in_=ones_col.rearrange("p (h x) -> p h x", x=1))

        qk_cm, qkp = openp(name="qk", bufs=16, side="right")
        w1_cm, w1 = openp(name="w1", bufs=3)
        acc_cm, accp = openp(name="psAC1", bufs=4, space="PSUM")
        qk_t = []
        for mc in range(8):
            wt = load_wp("attn_p", mc, w1)
            for mh in range(2):
                f = 2 * mc + mh
                qt = qkp.tile([P, T], BF16, tag="qk", name="qk")
                for tch in range(2):
                    sl = slice(512 * tch, 512 * (tch + 1))
                    aps = accp.tile([P, 512], F32, tag="acc", name="acc")
                    dr_group(aps, ws_passes(wt, None, x8, None,
                                            slice(128 * mh, 128 * mh + 128), sl))
                    nc.vector.tensor_tensor(out=qt[:, sl], in0=aps, in1=A_b[:, sl],
                                            op=OP.mult)
                    if qk_bias:
                        nc.vector.tensor_scalar_add(out=qt[:, sl], in0=qt[:, sl],
                                                    scalar1=bqk[:, f:f + 1])
                qk_t.append(qt)
        acc_cm.__exit__(None, None, None)
        w1_cm.__exit__(None, None, None)
        accv_cm.__exit__(None, None, None)
        wv_cm.__exit__(None, None, None)
        x8_cm.__exit__(None, None, None)

        # ================= P2: self attention =================
        o_cm, opool = openp(name="o1", bufs=1)
        o_all = opool.tile([P, NCT, T], BF16, tag="ot", name="ot")
        pp_cm, pp = openp(name="pp1", bufs=5)
        psS_cm, psS = openp(name="psS1", bufs=2, space="PSUM")

        def kq_self(h):
            return (qk_t[8 + h // 2], (h % 2) * D), (qk_t[h // 2], (h % 2) * D)

        for tch in range(2):
            for h in range(H):
                attn_chunk(kq_self, vaug, 4 * (tch + 1), h, tch, psS, pp,
                           causal=True, o_all=o_all, o_dt=BF16)

        psS_cm.__exit__(None, None, None)
        pp_cm.__exit__(None, None, None)
        qk_cm.__exit__(None, None, None)
        vap_cm.__exit__(None, None, None)

        # ================= P3: aproj (bf16) + residual in place ======
        w2_cm, w2 = openp(name="w2", bufs=3)
        acc_cm, accp = openp(name="psAC3", bufs=3, space="PSUM")
        for co in range(NCT):
            wt = load_wb("aproj_b", co, w2)
            for tch in range(2):
                sl = slice(512 * tch, 512 * (tch + 1))
                aps = accp.tile([P, 512], F32, tag="acc", name="acc")
                for c in range(NCT):
                    nc.tensor.matmul(aps, wt[:, c, :], o_all[:, c, sl],
                                     start=(c == 0), stop=(c == NCT - 1))
                nc.vector.tensor_tensor(out=resid[co][:, sl], in0=aps,
                                        in1=resid[co][:, sl], op=OP.add)
                if ab_bias:
                    nc.vector.tensor_scalar_add(
                        out=resid[co][:, sl], in0=resid[co][:, sl],
                        scalar1=bap_c[:, co:co + 1])
        acc_cm.__exit__(None, None, None)
        w2_cm.__exit__(None, None, None)
        o_cm.__exit__(None, None, None)

        # ================= P4: cross attention projections =================
        x1_cm, x1p = openp(name="x18", bufs=1)
        x18 = x1p.tile([P, NCT, T], F8, tag="x8", name="x8")

        ln_cm, lnp = openp(name="psLN1", bufs=2, space="PSUM")
        A_b, _ = ln_stats(resid, lnp)
        ln_cm.__exit__(None, None, None)
        quant_x(resid, x18, None)

        k2_cm, k2p = openp(name="k2", bufs=NCT, side="right")
        v2_cm, v2p = openp(name="v2", bufs=2, side="right")

        w3_cm, w3 = openp(name="w3", bufs=3)

        img_cm, imgp = openp(name="img", bufs=1)
        tok_cm, tokp = openp(name="tok4", bufs=2)
        tp_cm, tpp = openp(name="psT4", bufs=2, space="PSUM")
        imgT = imgp.tile([P, NCT, TI], F8, tag="imgT", name="imgT")
        for tt in range(TI // P):
            tok = tokp.tile([P, C], F32, tag="tok", name="tok")
            nc.sync.dma_start(out=tok, in_=dr["x_img_feats"].ap()[tt * P:(tt + 1) * P, :])
            for c in range(NCT):
                tps = tpp.tile([P, P], F32, tag="tp", name="tp")
                nc.tensor.transpose(tps, tok[:, c * P:(c + 1) * P], ident)
                nc.vector.tensor_copy(out=imgT[:, c, tt * P:(tt + 1) * P], in_=tps)
        tp_cm.__exit__(None, None, None)
        tok_cm.__exit__(None, None, None)

        acc_cm, accp = openp(name="psAC4", bufs=2, space="PSUM")
        k2_t = []
        for mc in range(4):
            wt = load_wp("k_p", mc, w3)
            for mh in range(2):
                f = 2 * mc + mh
                kt = k2p.tile([P, TI], BF16, tag="k2", name="k2")
                kps = accp.tile([P, 256], F32, tag="acc256", name="acc256")
                dr_group(kps, [(wt[:, kk, :, 128 * mh:128 * mh + 128],
                                imgT[:, 2 * kk:2 * kk + 2, :]) for kk in range(4)])
                nc.scalar.activation(out=kt, in_=kps, func=AF.Identity,
                                     bias=bk_c[:, f:f + 1], scale=WSI)
                k2_t.append(kt)

        brow_v2 = rowp.tile([1, C], F32, tag="row", name="braw2")
        nc.sync.dma_start(out=brow_v2, in_=dr["bv"].ap().rearrange("(a c) -> a c", a=1))
        wv2_cm, wv2 = openp(name="wv2", bufs=3)
        bvb2 = bcast_row(brow_v2, wv2, accp, "bvb2")

        v2aug = [v2p.tile([P, 16 * 65], F32R, tag="va2", name="va2")
                 for _ in range(TI // P)]
        for cc in range(4):
            wt = load_wp("v2_p", cc, wv2)
            for st in range(TI // P):
                vps = accp.tile([P, 256], F32, tag="acc256", name="acc256")
                dr_group(vps, [(imgT[:, 2 * kk:2 * kk + 2, st * P:(st + 1) * P],
                                wt[:, kk, :, :]) for kk in range(4)])
                dst = v2aug[st].rearrange("p (h x) -> p h x", x=65)[:, 4 * cc:4 * (cc + 1), 0:64]
                nc.vector.scalar_tensor_tensor(
                    out=dst, in0=vps.rearrange("p (h x) -> p h x", x=64),
                    scalar=WSI,
                    in1=bvb2[:, 256 * cc:256 * (cc + 1)].rearrange("p (h x) -> p h x", x=64),
                    op0=OP.mult, op1=OP.add)
        for st in range(TI // P):
            nc.vector.tensor_copy(
                out=v2aug[st].rearrange("p (h x) -> p h x", x=65)[:, :, 64:65],
                in_=ones_col.rearrange("p (h x) -> p h x", x=1))
        wv2_cm.__exit__(None, None, None)
        img_cm.__exit__(None, None, None)

        q2_cm, q2p = openp(name="q2", bufs=NCT, side="right")
        q2_t = []
        for mc in range(4):
            wt = load_wp("q_p", mc, w3)
            for mh in range(2):
                f = 2 * mc + mh
                qt = q2p.tile([P, T], BF16, tag="q2", name="q2")
                for tch in range(2):
                    sl = slice(512 * tch, 512 * (tch + 1))
                    aps = accp.tile([P, 512], F32, tag="acc", name="acc")
                    dr_group(aps, ws_passes(wt, None, x18, None,
                                            slice(128 * mh, 128 * mh + 128), sl))
                    nc.vector.tensor_tensor(out=qt[:, sl], in0=aps, in1=A_b[:, sl],
                                            op=OP.mult)
                    if q2_bias:
                        nc.vector.tensor_scalar_add(out=qt[:, sl], in0=qt[:, sl],
                                                    scalar1=bq_c[:, f:f + 1])
                q2_t.append(qt)
        acc_cm.__exit__(None, None, None)
        w3_cm.__exit__(None, None, None)
        x1_cm.__exit__(None, None, None)

        # ================= P5: cross attention =================
        o_cm, opool = openp(name="o2", bufs=1)
        o2_all = opool.tile([P, NCT, T], F8, tag="ot", name="ot")
        pp_cm, pp = openp(name="pp2", bufs=4)
        psS_cm, psS = openp(name="psS2", bufs=2, space="PSUM")

        def kq_cross(h):
            return (k2_t[h // 2], (h % 2) * D), (q2_t[h // 2], (h % 2) * D)

        for tch in range(2):
            for h in range(H):
                attn_chunk(kq_cross, v2aug, TI // P, h, tch, psS, pp,
                           causal=False, o_all=o2_all, o_dt=F8)

        psS_cm.__exit__(None, None, None)
        pp_cm.__exit__(None, None, None)
        q2_cm.__exit__(None, None, None)
        v2_cm.__exit__(None, None, None)
        k2_cm.__exit__(None, None, None)

        # ================= P6: cproj + residual (x2, in place) =================
        w4_cm, w4 = openp(name="w4", bufs=3)
        acc_cm, accp = openp(name="psAC5", bufs=3, space="PSUM")
        for mc in range(4):
            wt = load_wp("cproj_p", mc, w4)
            for mh in range(2):
                co = 2 * mc + mh
                for tch in range(2):
                    sl = slice(512 * tch, 512 * (tch + 1))
                    aps = accp.tile([P, 512], F32, tag="acc", name="acc")
                    dr_group(aps, ws_passes(wt, None, o2_all, None,
                                            slice(128 * mh, 128 * mh + 128), sl))
                    nc.vector.scalar_tensor_tensor(
                        out=resid[co][:, sl], in0=aps, scalar=WSI,
                        in1=resid[co][:, sl], op0=OP.mult, op1=OP.add)
                    if cp_bias:
                        nc.vector.tensor_scalar_add(
                            out=resid[co][:, sl], in0=resid[co][:, sl],
                            scalar1=bcp_c[:, co:co + 1])
        acc_cm.__exit__(None, None, None)
        w4_cm.__exit__(None, None, None)
        o_cm.__exit__(None, None, None)

        # ================= P7: MLP =================
        x2_cm, x2p = openp(name="x28", bufs=1)
        x28 = x2p.tile([P, NCT, T], F8, tag="x8", name="x8")
        x2d8 = x2p.tile([P, NCT, T], F8, tag="xd8", name="xd8")

        ln_cm, lnp = openp(name="psLN2", bufs=2, space="PSUM")
        A_b, _ = ln_stats(resid, lnp)
        ln_cm.__exit__(None, None, None)
        quant_x(resid, x28, x2d8)

        up_cm, up = openp(name="u", bufs=16, side="right")
        utiles = [up.tile([P, 2, T], BF16, tag="u", name="u") for _ in range(16)]
        uscr_cm, uscrp = openp(name="uscr", bufs=4)
        w5_cm, w5 = openp(name="w5", bufs=4)
        accU_cm, accU = openp(name="psU", bufs=4, space="PSUM")
        for mc in range(16):
            wt = load_wp("fc_p", mc, w5)
            wtR = load_wp("fcR_p", mc, w5)
            for mh in range(2):
                ff = 2 * mc + mh
                for tch in range(2):
                    sl = slice(512 * tch, 512 * (tch + 1))
                    ups = accU.tile([P, 512], F32, tag="acc", name="acc")
                    dr_group(ups, ws_passes(wt, wtR, x28, x2d8,
                                            slice(128 * mh, 128 * mh + 128), sl))
                    uscr = uscrp.tile([P, 512], F32, tag="us", name="us")
                    nc.vector.tensor_tensor(out=uscr, in0=ups, in1=A_b[:, sl],
                                            op=OP.mult)
                    nc.scalar.activation(out=utiles[ff // 2][:, ff % 2, sl], in_=uscr,
                                         func=AF.Gelu_apprx_tanh,
                                         bias=bfc_c[:, ff:ff + 1], scale=1.0)
        accU_cm.__exit__(None, None, None)
        w5_cm.__exit__(None, None, None)
        uscr_cm.__exit__(None, None, None)
        x2_cm.__exit__(None, None, None)

        w6_cm, w6 = openp(name="w6", bufs=3)
        psM_cm, psM = openp(name="psM", bufs=3, space="PSUM")
        for co in range(NCT):
            wt = load_wb("mproj_b", co, w6)
            for tch in range(2):
                sl = slice(512 * tch, 512 * (tch + 1))
                mps = psM.tile([P, 512], F32, tag="m", name="m")
                for ff in range(FF // P):
                    nc.tensor.matmul(mps, wt[:, ff, :], utiles[ff // 2][:, ff % 2, sl],
                                     start=(ff == 0), stop=(ff == FF // P - 1))
                nc.vector.tensor_tensor(out=resid[co][:, sl], in0=mps,
                                        in1=resid[co][:, sl], op=OP.add)
                if mp_bias:
                    nc.vector.tensor_scalar_add(
                        out=resid[co][:, sl], in0=resid[co][:, sl],
                        scalar1=bmp_c[:, co:co + 1])
        psM_cm.__exit__(None, None, None)
        w6_cm.__exit__(None, None, None)
        up_cm.__exit__(None, None, None)

        # ================= P8: transpose back & store =================
        tok_cm, tokp = openp(name="tok7", bufs=2)
        tp_cm, tpp = openp(name="psT7", bufs=4, space="PSUM")
        for tt in range(NTT):
            otok = tokp.tile([P, C], F32, tag="tok", name="tok")
            for c in range(NCT):
                tps = tpp.tile([P, P], F32R, tag="tpr", name="tpr")
                nc.tensor.transpose(tps, resid[c][:, tt * P:(tt + 1) * P], identR)
                if c % 2:
                    nc.vector.tensor_copy(out=otok[:, c * P:(c + 1) * P], in_=tps)
                else:
                    nc.scalar.copy(out=otok[:, c * P:(c + 1) * P], in_=tps)
            nc.sync.dma_start(out=out_d.ap()[tt * P:(tt + 1) * P, :], in_=otok)
        tp_cm.__exit__(None, None, None)
        tok_cm.__exit__(None, None, None)
        res_cm.__exit__(None, None, None)

        for cm in reversed(kw_cms):
            cm.__exit__(None, None, None)

    nc.compile()
    return nc


def _pack_core(Wq, Mc):
    K, M = Wq.shape
    KK, MC = K // 256, M // Mc
    A = Wq.reshape(KK, 2, P, MC, Mc).transpose(3, 2, 0, 1, 4)
    return np.ascontiguousarray(A.reshape(MC, P, KK * 2 * Mc))


def _pack_w(W, Mc=256):
    import ml_dtypes
    return _pack_core((np.asarray(W, np.float32) * WS).astype(ml_dtypes.float8_e4m3), Mc)


def _pack_wr(W, Mc=256):
    import ml_dtypes
    Ws = np.asarray(W, np.float32) * WS
    W8 = Ws.astype(ml_dtypes.float8_e4m3)
    R = Ws - W8.astype(np.float32)
    return _pack_core(R.astype(ml_dtypes.float8_e4m3), Mc)


def _pack_b(W):
    """[K, M] f32 -> [M//128, 128, (K//128)*128] bf16 stationary slabs."""
    import ml_dtypes
    K, M = W.shape
    A = np.asarray(W, np.float32).reshape(K // P, P, M // P, P).transpose(2, 1, 0, 3)
    return np.ascontiguousarray(A.reshape(M // P, P, K).astype(ml_dtypes.bfloat16))


def _fold(W, g):
    Wg = np.asarray(W, np.float32) * np.asarray(g, np.float32)[:, None]
    return Wg - Wg.mean(0, keepdims=True)


def kernel(**inputs):
    from concourse.bass_utils import run_bass_kernel_spmd

    np_inputs = {k: np.asarray(v, dtype=np.float32) for k, v in inputs.items()}
    g1, b1 = np_inputs["ln1_g"], np_inputs["ln1_b"]
    g2, b2 = np_inputs["ln2_g"], np_inputs["ln2_b"]
    Wa = np_inputs["W_attn"]

    # fold LN gains/means into weights; fold LN biases into effective biases
    Wa_f = _fold(Wa, g1)
    Wq_f = _fold(np_inputs["Wq"], g1)
    Wfc_f = _fold(np_inputs["W_fc"], g2)
    b_attn_eff = np_inputs["b_attn"] + Wa.T @ b1
    bq_eff = np_inputs["bq"] + np_inputs["Wq"].T @ b1
    bfc_eff = np_inputs["b_fc"] + np_inputs["W_fc"].T @ b2

    flags = (bool(np.any(b_attn_eff[0:2 * C])), bool(np.any(bq_eff)),
             bool(np.any(np_inputs["b_aproj"])), bool(np.any(np_inputs["bcproj"])),
             bool(np.any(np_inputs["b_mproj"])))
    key = ("nc", flags)
    if key not in _CACHED:
        _CACHED[key] = _build(flags)
    nc = _CACHED[key]

    packed = {
        "attn_p": _pack_w(Wa_f),
        "vR_p": _pack_wr(Wa_f[:, 2 * C:3 * C]),
        "q_p": _pack_w(Wq_f),
        "k_p": _pack_w(np_inputs["Wk"]),
        "v2_p": _pack_w(np_inputs["Wv"]),
        "cproj_p": _pack_w(np_inputs["Wcproj"]),
        "fc_p": _pack_w(Wfc_f),
        "fcR_p": _pack_wr(Wfc_f),
        "aproj_b": _pack_b(np_inputs["W_aproj"]),
        "mproj_b": _pack_b(np_inputs["W_mproj"]),
    }
    small = {
        "b_attn": b_attn_eff, "bq": bq_eff, "b_fc": bfc_eff,
        "b_aproj": np_inputs["b_aproj"], "bk": np_inputs["bk"],
        "bv": np_inputs["bv"], "bcproj": np_inputs["bcproj"],
        "b_mproj": np_inputs["b_mproj"],
    }
    in_maps = []
    for b in range(B):
        m = dict(small)
        m.update(packed)
        m["x"] = np.ascontiguousarray(np_inputs["x"][b])
        m["x_img_feats"] = np.ascontiguousarray(np_inputs["x_img_feats"][b])
        in_maps.append(m)
    res = run_bass_kernel_spmd(nc, in_maps, core_ids=list(range(B)))
    out = np.stack([res.results[b]["out"] for b in range(B)], axis=0)
    return out.astype(np.float32)
